# revision 1
# baseline (speedup 1.0000x reference)
"""MultiHeadLatentAttn TRN2 kernel (8 NeuronCores, uniform SPMD).

Sharding:
  Phase A (token-parallel): core c (batch b=c//4, j=c%4) owns tokens
  [j*512,(j+1)*512) of batch b. Computes latentT, kT, pos_kT for its
  tokens; AllGather(latent) and AllGather(k,pos_k) within each batch
  group of 4 cores.
  Phase B (head-parallel): core owns heads 4j..4j+3. Computes qT, pos_qT,
  v for its heads over ALL tokens from the gathered latent; causal
  attention for 4 heads; row-parallel o_proj giving a partial output
  [2048 model, 2048 tokens].
  Host: sums the 4 partials per batch, transposes, adds b_o.

All matmuls run as float32r (full PE rate at free-dim>=256, ~1e-4 rel err).
"""

import os
import sys

import numpy as np

for _p in ("/opt/trn_rl_repo", "/root/.axon_site/_ro/trn_rl_repo"):
    if os.path.isdir(_p) and _p not in sys.path:
        sys.path.append(_p)

import concourse.bass as bass
import concourse.mybir as mybir
import concourse.tile as tile
from concourse import bacc
from concourse import bass_utils

F32 = mybir.dt.float32
F32R = mybir.dt.bfloat16  # matmul-path dtype (was float32r)

MODEL = 2048
LATENT = 512
L3 = 3 * LATENT            # 1536
POS = 1024
NH = 16
HD = 128                   # head dim
PHD = 64                   # pos head dim
DC = HD + PHD              # 192
B, S = 2, 2048
TOK = 512                  # tokens per core
NCORES = 8
ROPE_THETA = 50000.0
SCALE = 1.0 / float(np.sqrt(DC))

MC = MODEL // 128          # 16 model-dim chunks
LC = L3 // 128             # 12 latent3 chunks
LQC = LATENT // 128        # 4 latent_q chunks

RG = [[0, 1, 2, 3], [4, 5, 6, 7]]
F32_INPUTS = {"bd", "bk", "bq", "bqp", "bkp"}


def _emit(nc, tc, T):
    """Emit the whole uniform SPMD program. T: dict of dram tensor APs."""
    from contextlib import ExitStack
    Ex = mybir.ActivationFunctionType.Exp
    Ident = mybir.ActivationFunctionType.Identity

    with tc.tile_pool(name="dram", bufs=1, space="DRAM") as dram:
        lat_cs = [dram.tile([512, TOK], F32R, name=f"lat_c{g}")
                  for g in range(3)]
        lat_gs = [dram.tile([4, 512, TOK], F32R, name=f"lat_g{g}")
                  for g in range(3)]
        k_c = dram.tile([MODEL + PHD, TOK], F32R, name="k_c")
        k_g = dram.tile([4, MODEL + PHD, TOK], F32R, name="k_g")

        persist_ctx = ExitStack()
        persistp = persist_ctx.enter_context(
            tc.tile_pool(name="persist", bufs=1))
        qt = [persistp.tile([128, S], F32R, name=f"qt{hi}", tag=f"qt{hi}")
              for hi in range(4)]
        pq = [persistp.tile([PHD, S], F32R, name=f"pq{hi}", tag=f"pq{hi}")
              for hi in range(4)]
        vt = [persistp.tile([128, 512], F32R, name=f"vt{tt}", tag=f"vt{tt}")
              for tt in range(16)]
        attn = [persistp.tile([128, S], F32R, name=f"attn{hi}",
                              tag=f"at{hi}")
                for hi in range(4)]

        with tc.tile_pool(name="constA", bufs=1) as cA:
            cosq = cA.tile([128, S], F32R, name="cosq")
            sinq = cA.tile([128, S], F32R, name="sinq")
            cosk = cA.tile([PHD, TOK], F32R, name="cosk")
            sink = cA.tile([PHD, TOK], F32R, name="sink")
            bd = cA.tile([128, LC], F32, name="bd")
            bk = cA.tile([128, MC], F32, name="bk")
            bq = cA.tile([128, 4], F32, name="bq")
            bqp = cA.tile([128, 2], F32, name="bqp")
            bkp = cA.tile([PHD, 1], F32, name="bkp")
            bv = cA.tile([1, 512], F32R, name="bv")
            ones1 = cA.tile([1, 128], F32R, name="ones1")
            nc.sync.dma_start(cosq[:], T["cosq"][:])
            nc.sync.dma_start(sinq[:], T["sinq"][:])
            nc.sync.dma_start(cosk[:], T["cosk"][:])
            nc.sync.dma_start(sink[:], T["sink"][:])
            nc.sync.dma_start(bd[:], T["bd"][:])
            nc.sync.dma_start(bk[:], T["bk"][:])
            nc.sync.dma_start(bq[:], T["bq"][:])
            nc.sync.dma_start(bqp[:], T["bqp"][:])
            nc.sync.dma_start(bkp[:], T["bkp"][:])
            nc.sync.dma_start(bv[:], T["bv"][:])
            nc.sync.dma_start(ones1[:], T["ones1"][:])

            wres_ctx = ExitStack()
            wres = wres_ctx.enter_context(tc.tile_pool(name="wres", bufs=1))
            # resident B1 weights (each tile reused by many matmuls)
            wuq_r, wuv_r, wqp_r = [], [], []
            for lc in range(LC):
                t = wres.tile([128, 512], F32R, name=f"wuqr{lc}",
                              tag=f"wuqr{lc}")
                nc.sync.dma_start(
                    t[:], T["Wuq"][lc * 128:(lc + 1) * 128, :])
                wuq_r.append(t)
                t = wres.tile([128, 512], F32R, name=f"wuvr{lc}",
                              tag=f"wuvr{lc}")
                nc.sync.dma_start(
                    t[:], T["Wuv"][lc * 128:(lc + 1) * 128, :])
                wuv_r.append(t)
            for lc in range(LQC):
                t = wres.tile([128, 256], F32R, name=f"wqpr{lc}",
                              tag=f"wqpr{lc}")
                nc.sync.dma_start(
                    t[:], T["Wqp"][lc * 128:(lc + 1) * 128, :])
                wqp_r.append(t)


            # ---------------- Phase A: token-local projections ----------
            with (
                tc.tile_pool(name="xa", bufs=1) as xap,
                tc.tile_pool(name="latp", bufs=1) as latp,
                tc.tile_pool(name="wstr", bufs=6) as wstr,
                tc.tile_pool(name="evA", bufs=3) as evA,
                tc.tile_pool(name="psA", bufs=1, space="PSUM") as psA,
            ):
                # prefetch pos_k weights first (first matmuls need them)
                wkp_t = []
                for mc in range(MC):
                    w = wstr.tile([128, PHD], F32R, name=f"wkp{mc}",
                                  tag=f"wkp{mc}")
                    nc.sync.dma_start(
                        w[:], T["Wkp"][mc * 128:(mc + 1) * 128, :])
                    wkp_t.append(w)
                xa = []
                for mc in range(MC):
                    t = xap.tile([128, TOK], F32R, name=f"xa{mc}", tag=f"xa{mc}")
                    nc.sync.dma_start(t[:], T["xT"][mc * 128:(mc + 1) * 128, :])
                    xa.append(t)

                # pos_kT [64, 512] + rope
                psk = psA.tile([PHD, TOK], F32, name="pspk", tag="ps0")
                for mc in range(MC):
                    nc.tensor.matmul(psk[:], wkp_t[mc][:], xa[mc][:],
                                     start=(mc == 0), stop=(mc == MC - 1))
                pkraw = evA.tile([PHD, TOK], F32R, name="pkraw", tag="pkraw")
                nc.scalar.activation(pkraw[:], psk[:], Ident, bias=bkp[:, 0:1])
                pk1 = evA.tile([PHD, TOK], F32R, name="pk1", tag="pk1")
                pku = evA.tile([PHD, TOK], F32R, name="pku", tag="pku")
                pkr = evA.tile([PHD, TOK], F32R, name="pkr", tag="pkr")
                nc.vector.tensor_mul(pk1[:], pkraw[:], cosk[:])
                nc.vector.tensor_mul(pku[:], pkraw[:], sink[:])
                nc.sync.dma_start(pkr[0:32, :], pku[32:64, :])
                nc.sync.dma_start(pkr[32:64, :], pku[0:32, :])
                nc.vector.tensor_add(pk1[:], pk1[:], pkr[:])
                nc.sync.dma_start(k_c[MODEL:MODEL + PHD, :], pk1[:])

                # latentT [1536, 512] in two psum passes (8 + 4)
                lat = [
                    latp.tile([128, TOK], F32R, name=f"lat{lt}", tag=f"lat{lt}")
                    for lt in range(LC)
                ]
                for ltg in ((0, 8), (8, 12)):
                    lo, hi = ltg
                    ps = [psA.tile([128, TOK], F32, name=f"psl{lt}", tag=f"ps{i}")
                          for i, lt in enumerate(range(lo, hi))]
                    for mc in range(MC):
                        w = wstr.tile([128, (hi - lo) * 128], F32R,
                                      name=f"wd{lo}_{mc}", tag="w")
                        nc.sync.dma_start(
                            w[:], T["Wd"][mc * 128:(mc + 1) * 128, lo * 128:hi * 128])
                        for i, lt in enumerate(range(lo, hi)):
                            nc.tensor.matmul(
                                ps[i][:], w[:, i * 128:(i + 1) * 128], xa[mc][:],
                                start=(mc == 0), stop=(mc == MC - 1))
                    for i, lt in enumerate(range(lo, hi)):
                        nc.scalar.activation(lat[lt][:], ps[i][:], Ident,
                                             bias=bd[:, lt:lt + 1])
                        g, r = divmod(lt, 4)
                        nc.sync.dma_start(
                            lat_cs[g][r * 128:(r + 1) * 128, :], lat[lt][:])
                        if lt % 4 == 3:
                            nc.gpsimd.collective_compute(
                                "AllGather", mybir.AluOpType.bypass,
                                replica_groups=RG,
                                ins=[lat_cs[g][:]], outs=[lat_gs[g][:]])

                # kT [2048, 512] in two psum passes of 8 tiles
                for dtg in ((0, 8), (8, 16)):
                    lo, hi = dtg
                    ps = [psA.tile([128, TOK], F32, name=f"psk{dt}", tag=f"ps{i}")
                          for i, dt in enumerate(range(lo, hi))]
                    for lc in range(LC):
                        w = wstr.tile([128, (hi - lo) * 128], F32R,
                                      name=f"wuk{lo}_{lc}", tag="w")
                        nc.sync.dma_start(
                            w[:], T["Wuk"][lc * 128:(lc + 1) * 128, lo * 128:hi * 128])
                        for i, dt in enumerate(range(lo, hi)):
                            nc.tensor.matmul(
                                ps[i][:], w[:, i * 128:(i + 1) * 128], lat[lc][:],
                                start=(lc == 0), stop=(lc == LC - 1))
                    for i, dt in enumerate(range(lo, hi)):
                        kt = evA.tile([128, TOK], F32R, name=f"kt{dt}", tag="kt")
                        nc.scalar.activation(kt[:], ps[i][:], Ident,
                                             bias=bk[:, dt:dt + 1])
                        nc.sync.dma_start(k_c[dt * 128:(dt + 1) * 128, :], kt[:])

                nc.gpsimd.collective_compute(
                    "AllGather", mybir.AluOpType.bypass, replica_groups=RG,
                    ins=[k_c[:]], outs=[k_g[:]])

            # ---------------- Phase B1: head-local q/pos_q/v ------------
            with (
                tc.tile_pool(name="latg", bufs=1) as latgp,
                tc.tile_pool(name="ropet", bufs=2) as ropet,
                tc.tile_pool(name="psB", bufs=1, space="PSUM") as psB,
            ):
                latg = []
                for lc in range(LC):
                    t = latgp.tile([128, S], F32R, name=f"latg{lc}",
                                   tag=f"latg{lc}")
                    g, r = divmod(lc, 4)
                    nc.gpsimd.dma_start(
                        t[:].rearrange("p (c t) -> p c t", c=4),
                        lat_gs[g][:, r * 128:(r + 1) * 128, :].rearrange(
                            "c p t -> p c t"))
                    latg.append(t)

                # qT for my 4 heads -> qt (SBUF resident)
                for hi in range(4):
                    for tcn in range(4):
                        ps = psB.tile([128, 512], F32, name=f"psq{hi}{tcn}",
                                      tag=f"psb{tcn}")
                        for lc in range(LC):
                            nc.tensor.matmul(
                                ps[:], wuq_r[lc][:, hi * 128:(hi + 1) * 128],
                                latg[lc][:, tcn * 512:(tcn + 1) * 512],
                                start=(lc == 0), stop=(lc == LC - 1))
                        nc.scalar.activation(
                            qt[hi][:, tcn * 512:(tcn + 1) * 512], ps[:],
                            Ident, bias=bq[:, hi:hi + 1])

                # pos_qT my slice + rope -> pq (4 x [64, S])
                for pi in range(2):
                    for tcn in range(4):
                        ps = psB.tile([128, 512], F32, name=f"pspq{pi}{tcn}",
                                      tag=f"psb{pi * 4 + tcn % 4}")
                        for lc in range(LQC):
                            nc.tensor.matmul(
                                ps[:], wqp_r[lc][:, pi * 128:(pi + 1) * 128],
                                latg[lc][:, tcn * 512:(tcn + 1) * 512],
                                start=(lc == 0), stop=(lc == LQC - 1))
                        raw = ropet.tile([128, 512], F32R,
                                         name=f"pqr{pi}{tcn}", tag="praw")
                        nc.scalar.activation(raw[:], ps[:], Ident,
                                             bias=bqp[:, pi:pi + 1])
                        cs = slice(tcn * 512, (tcn + 1) * 512)
                        t1 = ropet.tile([128, 512], F32R, name=f"t1{pi}{tcn}",
                                        tag="t1")
                        tu = ropet.tile([128, 512], F32R, name=f"tu{pi}{tcn}",
                                        tag="tu")
                        tr = ropet.tile([128, 512], F32R, name=f"tr{pi}{tcn}",
                                        tag="tr")
                        nc.vector.tensor_mul(t1[:], raw[:], cosq[:, cs])
                        nc.vector.tensor_mul(tu[:], raw[:], sinq[:, cs])
                        for h2 in range(2):
                            o = h2 * 64
                            nc.sync.dma_start(tr[o:o + 32, :],
                                              tu[o + 32:o + 64, :])
                            nc.sync.dma_start(tr[o + 32:o + 64, :],
                                              tu[o:o + 32, :])
                        nc.vector.tensor_add(t1[:], t1[:], tr[:])
                        nc.sync.dma_start(pq[2 * pi][:, cs], t1[0:64, :])
                        nc.sync.dma_start(pq[2 * pi + 1][:, cs],
                                          t1[64:128, :])

                # v my head cols, natural layout -> vt (SBUF resident)
                for ttg in ((0, 8), (8, 16)):
                    lo, hi = ttg
                    ps = [psB.tile([128, 512], F32, name=f"psv{tt}",
                                   tag=f"psb{i}")
                          for i, tt in enumerate(range(lo, hi))]
                    for lc in range(LC):
                        for i, tt in enumerate(range(lo, hi)):
                            nc.tensor.matmul(
                                ps[i][:],
                                latg[lc][:, tt * 128:(tt + 1) * 128],
                                wuv_r[lc][:],
                                start=(lc == 0), stop=False)
                    for i, tt in enumerate(range(lo, hi)):
                        nc.tensor.matmul(ps[i][:], ones1[:], bv[:],
                                         start=False, stop=True)
                        nc.scalar.copy(vt[tt][:], ps[i][:])

            wres_ctx.close()

        # ---------------- Phase B2: attention -----------------------
        if True:
            with (
                tc.tile_pool(name="constB", bufs=1) as cB,
                tc.tile_pool(name="kvq", bufs=1) as kvq,
                tc.tile_pool(name="ep", bufs=3) as ep,
                tc.tile_pool(name="e2p", bufs=2) as e2p,
                tc.tile_pool(name="rcp", bufs=2) as rcp,
                tc.tile_pool(name="pss", bufs=3, space="PSUM") as pss,
                tc.tile_pool(name="psav", bufs=2, space="PSUM") as psav,
                tc.tile_pool(name="psden", bufs=2, space="PSUM") as psden,
            ):
                masks = []
                for m in range(4):
                    t = cB.tile([128, 512], F32R, name=f"mask{m}")
                    nc.sync.dma_start(t[:], T[f"mask{m}"][:])
                    masks.append(t)
                ONES = cB.tile([128, 128], F32R, name="ONES")
                nc.sync.dma_start(ONES[:], T["ONES"][:])

                pid = nc.gpsimd.partition_id()
                roff = (pid % 4) * 512

                kct = []
                for hi in range(4):
                    t = kvq.tile([128, S], F32R, name=f"kct{hi}",
                                 tag=f"kct{hi}")
                    nc.gpsimd.dma_start(
                        t[:].rearrange("p (c t) -> p c t", c=4),
                        k_g[:, bass.ds(roff + hi * 128, 128), :].rearrange(
                            "c p t -> p c t"))
                    kct.append(t)
                pk = kvq.tile([PHD, S], F32R, name="pk", tag="pk")
                nc.gpsimd.dma_start(
                    pk[:].rearrange("p (c t) -> p c t", c=4),
                    k_g[:, MODEL:MODEL + PHD, :].rearrange("c p t -> p c t"))
                for h in range(4):
                    for qB in range(4):
                        qs = slice(qB * 512, (qB + 1) * 512)
                        nkt = 4 * qB + 4
                        av = psav.tile([128, 512], F32, name=f"av{h}{qB}",
                                       tag="av")
                        den = psden.tile([128, 512], F32,
                                         name=f"den{h}{qB}", tag="den")
                        for kt in range(nkt):
                            ks = slice(kt * 128, (kt + 1) * 128)
                            sps = pss.tile([128, 512], F32,
                                           name=f"s{h}{qB}{kt}", tag="s")
                            nc.tensor.matmul(sps[:], kct[h][:, ks],
                                             qt[h][:, qs],
                                             start=True, stop=False)
                            nc.tensor.matmul(sps[:], pk[:, ks],
                                             pq[h][:, qs],
                                             start=False, stop=True)
                            e = ep.tile([128, 512], F32R,
                                        name=f"e{h}{qB}{kt}", tag="e")
                            nc.scalar.activation(e[:], sps[:], Ex,
                                                 scale=SCALE)
                            m = kt - 4 * qB
                            if m >= 0:
                                e2 = e2p.tile([128, 512], F32R,
                                              name=f"e2_{h}{qB}{kt}",
                                              tag="e2")
                                nc.vector.tensor_mul(e2[:], e[:],
                                                     masks[m][:])
                                e = e2
                            nc.tensor.matmul(den[:], ONES[:], e[:],
                                             start=(kt == 0),
                                             stop=(kt == nkt - 1))
                            nc.tensor.matmul(
                                av[:], vt[kt][:, h * 128:(h + 1) * 128],
                                e[:], start=(kt == 0),
                                stop=(kt == nkt - 1))
                        rc = rcp.tile([128, 512], F32, name=f"rc{h}{qB}",
                                      tag="rc")
                        nc.vector.reciprocal(rc[:], den[:])
                        nc.vector.tensor_mul(attn[h][:, qs], av[:], rc[:])

            # ---------------- Phase C: o_proj partial ---------------
            with (
                tc.tile_pool(name="wop", bufs=1) as wop,
                tc.tile_pool(name="evC", bufs=4) as evC,
                tc.tile_pool(name="psC", bufs=1, space="PSUM") as psC,
            ):
                wos = []
                for hc in range(4):
                    t = wop.tile([128, MODEL], F32R, name=f"wos{hc}",
                                 tag=f"wos{hc}")
                    nc.sync.dma_start(
                        t[:], T["WoS"][hc * 128:(hc + 1) * 128, :])
                    wos.append(t)
                for tcn in range(4):
                    for mtp in range(2):
                        ps = [psC.tile([128, 512], F32,
                                       name=f"pso{tcn}{mtp}{i}",
                                       tag=f"psc{i}")
                              for i in range(8)]
                        for hc in range(4):
                            for i in range(8):
                                mt = mtp * 8 + i
                                nc.tensor.matmul(
                                    ps[i][:],
                                    wos[hc][:, mt * 128:(mt + 1) * 128],
                                    attn[hc][:, tcn * 512:(tcn + 1) * 512],
                                    start=(hc == 0), stop=(hc == 3))
                        for i in range(8):
                            mt = mtp * 8 + i
                            oe = evC.tile([128, 512], F32,
                                          name=f"oe{tcn}{mtp}{i}", tag="oe")
                            nc.scalar.copy(oe[:], ps[i][:])
                            nc.sync.dma_start(
                                T["OT"][mt * 128:(mt + 1) * 128,
                                        tcn * 512:(tcn + 1) * 512], oe[:])
        persist_ctx.close()


def build_program():
    nc = bacc.Bacc("TRN2", target_bir_lowering=False, debug=False,
                   num_devices=NCORES)
    T = {}

    def inp(name, shape, dt=F32R):
        T[name] = nc.dram_tensor(name, shape, dt, kind="ExternalInput").ap()

    inp("xT", [MODEL, TOK])
    inp("Wd", [MODEL, L3])
    inp("Wuk", [L3, MODEL])
    inp("Wuq", [L3, 512])
    inp("Wuv", [L3, 512])
    inp("Wqp", [LATENT, 256])
    inp("Wkp", [MODEL, PHD])
    inp("WoS", [512, MODEL])
    inp("cosq", [128, S])
    inp("sinq", [128, S])
    inp("cosk", [PHD, TOK])
    inp("sink", [PHD, TOK])
    inp("bd", [128, LC], F32)
    inp("bk", [128, MC], F32)
    inp("bq", [128, 4], F32)
    inp("bqp", [128, 2], F32)
    inp("bkp", [PHD, 1], F32)
    inp("bv", [1, 512])
    inp("ones1", [1, 128])
    inp("ONES", [128, 128])
    for m in range(4):
        inp(f"mask{m}", [128, 512])
    T["OT"] = nc.dram_tensor("OT", [MODEL, S], F32, kind="ExternalOutput").ap()

    with tile.TileContext(nc) as tc:
        _emit(nc, tc, T)
    nc.compile()
    return nc


def host_inputs(inputs):
    """Build the 8 per-core input maps from the full problem inputs."""
    import ml_dtypes
    bf16 = ml_dtypes.bfloat16
    x = np.ascontiguousarray(np.asarray(inputs["x"], np.float32))
    W_down = np.asarray(inputs["W_down"], np.float32)
    b_down = np.asarray(inputs["b_down"], np.float32)
    W_up = np.asarray(inputs["W_up"], np.float32)
    b_up = np.asarray(inputs["b_up"], np.float32)
    W_qpos = np.asarray(inputs["W_qpos"], np.float32)
    b_qpos = np.asarray(inputs["b_qpos"], np.float32)
    W_kpos = np.asarray(inputs["W_kpos"], np.float32)
    b_kpos = np.asarray(inputs["b_kpos"], np.float32)
    W_o = np.asarray(inputs["W_o"], np.float32)

    inv = (1.0 / ROPE_THETA ** (np.arange(0, PHD, 2, dtype=np.float32) / PHD))
    t_all = np.arange(S, dtype=np.float32)
    fr = np.outer(inv, t_all)                       # [32, S]
    cc = np.concatenate([np.cos(fr), np.cos(fr)], 0)        # [64, S]
    ss = np.sin(fr)
    # pre-signed for rotate-via-DMA: rows 0:32 -> +sin, rows 32:64 -> -sin
    ssn = np.concatenate([ss, -ss], 0)                      # [64, S]
    cosq = np.vstack([cc, cc]).astype(np.float32)           # [128, S]
    sinq = np.vstack([ssn, ssn]).astype(np.float32)

    qq = np.arange(512)[None, :]
    kk = np.arange(128)[:, None]
    masks = {
        f"mask{m}": np.where(qq >= kk + m * 128, 1.0, 0.0).astype(np.float32)
        for m in range(4)
    }

    common = {
        "Wd": np.ascontiguousarray(W_down),
        "Wuk": np.ascontiguousarray(W_up[:, MODEL:2 * MODEL]),
        "Wkp": np.ascontiguousarray(W_kpos),
        "cosq": cosq, "sinq": sinq,
        "bd": np.ascontiguousarray(b_down.reshape(LC, 128).T),
        "bk": np.ascontiguousarray(b_up[MODEL:2 * MODEL].reshape(MC, 128).T),
        "bkp": np.ascontiguousarray(b_kpos[:, None]),
        "ones1": np.ones((1, 128), np.float32),
        "ONES": np.ones((128, 128), np.float32),
        **masks,
    }
    maps = []
    for c in range(NCORES):
        b, j = divmod(c, 4)
        ts = slice(j * TOK, (j + 1) * TOK)
        hs = slice(j * 512, (j + 1) * 512)        # my 4 heads' flat dims
        m = dict(common)
        m["xT"] = np.ascontiguousarray(x[b, ts, :].T)
        m["Wuq"] = np.ascontiguousarray(W_up[:, :MODEL][:, hs])
        m["Wuv"] = np.ascontiguousarray(W_up[:, 2 * MODEL:][:, hs])
        m["Wqp"] = np.ascontiguousarray(W_qpos[:, j * 256:(j + 1) * 256])
        m["WoS"] = np.ascontiguousarray(W_o[hs, :])
        m["cosk"] = np.ascontiguousarray(cc[:, ts])
        m["sink"] = np.ascontiguousarray(ssn[:, ts])
        m["bq"] = np.ascontiguousarray(b_up[:MODEL][hs].reshape(4, 128).T)
        m["bqp"] = np.ascontiguousarray(
            b_qpos[j * 256:(j + 1) * 256].reshape(2, 128).T)
        m["bv"] = np.ascontiguousarray(b_up[2 * MODEL:][hs][None, :])
        for key in list(m):
            if key not in F32_INPUTS:
                m[key] = np.ascontiguousarray(m[key]).astype(bf16)
        maps.append(m)
    return maps


_NC_CACHE = None


def _program():
    global _NC_CACHE
    if _NC_CACHE is None:
        _NC_CACHE = build_program()
    return _NC_CACHE


def kernel(**inputs) -> np.ndarray:
    nc = _program()
    maps = host_inputs(inputs)
    kwargs = {}
    if os.environ.get("BASSK_TRACE"):
        kwargs = dict(trace=True, trace_cores=list(range(NCORES)))
        td = os.environ.get("BASSK_TRACE_DIR")
        if td:
            kwargs["tmpdir"] = td
    res = bass_utils.run_bass_kernel_spmd(
        nc, maps, core_ids=list(range(NCORES)), **kwargs)
    kernel.last_results = res
    b_o = np.asarray(inputs["b_o"], np.float32)
    out = np.empty((B, S, MODEL), np.float32)
    for b in range(B):
        acc = res.results[b * 4]["OT"].copy()
        for c in range(b * 4 + 1, b * 4 + 4):
            acc += res.results[c]["OT"]
        out[b] = acc.T + b_o[None, :]
    return out



# revision 8
# speedup vs baseline: 1.1466x; 1.1466x over previous
"""MultiHeadLatentAttn TRN2 kernel (8 NeuronCores, uniform SPMD). v2.

Sharding:
  Phase A (token-parallel): core c (batch b=c//4, j=c%4) owns tokens
  [j*512,(j+1)*512) of batch b. Computes latentT [1536,512] + rope'd
  pos_kT [64,512]; AllGathers them within the batch group of 4 cores in
  3 pipelined chunks (g0/g1/g2+pos_k).
  Phase B1 (head-parallel): from the gathered latent each core computes
  qT, kT (its 4 heads only -- no k AllGather), pos_qT, v over ALL
  tokens, chunk-major so PSUM stays within 8 banks.
  Phase B2: causal attention for 4 heads, qB-outer; paired [128,1024]
  exp activations; fast-approx reciprocal; row-parallel o_proj per
  query block; bf16 partial output [2048 model, 2048 tokens].
  Host: sums the 4 partials per batch, transposes, adds b_o.
"""

import os
import sys

import numpy as np

for _p in ("/opt/trn_rl_repo", "/root/.axon_site/_ro/trn_rl_repo"):
    if os.path.isdir(_p) and _p not in sys.path:
        sys.path.append(_p)

import concourse.bass as bass
import concourse.mybir as mybir
import concourse.tile as tile
from concourse import bacc
from concourse import bass_utils

F32 = mybir.dt.float32
BF16 = mybir.dt.bfloat16

MODEL = 2048
LATENT = 512
L3 = 3 * LATENT            # 1536
NH = 16
HD = 128
PHD = 64
DC = HD + PHD              # 192
B, S = 2, 2048
TOK = 512                  # tokens per core in phase A
NCORES = 8
ROPE_THETA = 50000.0
SCALE = 1.0 / float(np.sqrt(DC))

MC = MODEL // 128          # 16
LC = L3 // 128             # 12
LQC = LATENT // 128        # 4

RG = [[0, 1, 2, 3], [4, 5, 6, 7]]
F32_INPUTS = {"bd", "bk", "bq", "bqp", "bkp"}


def _emit(nc, tc, T):
    from contextlib import ExitStack
    Ex = mybir.ActivationFunctionType.Exp
    Ident = mybir.ActivationFunctionType.Identity

    with tc.tile_pool(name="dram", bufs=1, space="DRAM") as dram:
        lat_cs = [dram.tile([512, TOK], BF16, name=f"lat_c{g}")
                  for g in range(2)]
        lat_cs.append(dram.tile([512 + PHD, TOK], BF16, name="lat_c2"))
        lat_gs = [dram.tile([4, 512, TOK], BF16, name=f"lat_g{g}")
                  for g in range(2)]
        lat_gs.append(dram.tile([4, 512 + PHD, TOK], BF16, name="lat_g2"))

        persist_ctx = ExitStack()
        persistp = persist_ctx.enter_context(
            tc.tile_pool(name="persist", bufs=1))
        qt = [persistp.tile([128, S], BF16, name=f"qt{h}", tag=f"qt{h}")
              for h in range(4)]
        kct = [persistp.tile([128, S], BF16, name=f"kct{h}", tag=f"kc{h}")
               for h in range(4)]
        vt = [persistp.tile([128, 512], BF16, name=f"vt{tt}", tag=f"vt{tt}")
              for tt in range(16)]
        pq = [persistp.tile([PHD, S], BF16, name=f"pq{h}", tag=f"pq{h}")
              for h in range(4)]
        pk = persistp.tile([PHD, S], BF16, name="pk", tag="pk")
        attn = [persistp.tile([128, S], BF16, name=f"attn{h}", tag=f"at{h}")
                for h in range(4)]
        wos_r = [persistp.tile([128, MODEL], BF16, name=f"wos{hc}",
                               tag=f"wos{hc}") for hc in range(4)]
        maskp = [persistp.tile([128, 1024], BF16, name=f"maskp{m}",
                               tag=f"mp{m}") for m in range(2)]
        bk = persistp.tile([128, 4], F32, name="bk", tag="bk")
        bq = persistp.tile([128, 4], F32, name="bq", tag="bq")
        bqp = persistp.tile([128, 2], F32, name="bqp", tag="bqp")
        bv = persistp.tile([1, 512], BF16, name="bv", tag="bv")
        ones1 = persistp.tile([1, 128], BF16, name="ones1", tag="ones1")
        ONES = persistp.tile([128, 128], BF16, name="ONES", tag="ONES")
        dume = persistp.tile([128, 1], BF16, name="dume", tag="dume")

        nc.vector.memset(ones1[:], 1.0)
        nc.vector.memset(ONES[:], 1.0)
        nc.vector.memset(dume[:], 0.0)

        # first lc-group of B1 weights: loaded during phase A, freed after B1
        wug0_ctx = ExitStack()
        wug0p = wug0_ctx.enter_context(tc.tile_pool(name="wug0", bufs=1))
        wuq0 = wug0p.tile([128, 2048], BF16, name="wuq0", tag="wuq0")
        wuk0 = wug0p.tile([128, 2048], BF16, name="wuk0", tag="wuk0")
        wuv0 = wug0p.tile([128, 2048], BF16, name="wuv0", tag="wuv0")
        wqp_r = wug0p.tile([128, 1024], BF16, name="wqp_r", tag="wqp")

        # ---------------- Phase A ----------------
        with (
            tc.tile_pool(name="xa", bufs=1) as xap,
            tc.tile_pool(name="wdp", bufs=1) as wdp,
            tc.tile_pool(name="latp", bufs=1) as latp,
            tc.tile_pool(name="cA", bufs=1) as cA,
            tc.tile_pool(name="evA", bufs=1) as evA,
            tc.tile_pool(name="psA", bufs=1, space="PSUM") as psA,
            tc.tile_pool(name="psPK", bufs=1, space="PSUM") as psPK,
        ):
            # x and Wd as a few big batched DMAs (sync HWDGE ring)
            xa_all = xap.tile([128, MC * TOK], BF16, name="xa_all",
                              tag="xa")
            nc.sync.dma_start(
                xa_all[:].rearrange("p (c t) -> p c t", c=MC),
                T["xT"][:, :].rearrange("(c p) t -> p c t", c=MC))
            wdg = []
            for g in range(3):
                w = wdp.tile([128, MC * 512], BF16, name=f"wdg{g}",
                             tag=f"wdg{g}")
                nc.sync.dma_start(
                    w[:].rearrange("p (c f) -> p c f", c=MC),
                    T["Wd"][:, g * 512:(g + 1) * 512].rearrange(
                        "(c p) f -> p c f", c=MC))
                wdg.append(w)

            def xat(mc):
                return xa_all[:, mc * TOK:(mc + 1) * TOK]

            bd = cA.tile([128, LC], F32, name="bd", tag="bd")
            bkp = cA.tile([PHD, 1], F32, name="bkp", tag="bkp")
            wkp = cA.tile([128, MC * PHD], BF16, name="wkp", tag="wkp")
            cosk = cA.tile([PHD, TOK], BF16, name="cosk", tag="cosk")
            sink = cA.tile([PHD, TOK], BF16, name="sink", tag="sink")
            nc.sync.dma_start(bd[:], T["bd"][:])
            nc.sync.dma_start(bkp[:], T["bkp"][:])
            nc.sync.dma_start(
                wkp[:].rearrange("p (c f) -> p c f", c=MC),
                T["Wkp"][:, :].rearrange("(c p) f -> p c f", c=MC))
            nc.sync.dma_start(cosk[:], T["cosk"][:])
            nc.sync.dma_start(sink[:], T["sink"][:])

            # B1/B2 weights prefetch on the gpsimd (SWDGE) queue: keeps the
            # scalar queue free for ACT work and sync free for phase A.
            nc.gpsimd.dma_start(wuq0[:].rearrange("p (c f) -> p c f", c=4),
                                T["Wuq"][0:512, :].rearrange(
                                    "(c p) f -> p c f", c=4))
            nc.gpsimd.dma_start(wuk0[:].rearrange("p (c f) -> p c f", c=4),
                                T["Wuk"][0:512, :].rearrange(
                                    "(c p) f -> p c f", c=4))
            nc.gpsimd.dma_start(wuv0[:].rearrange("p (c f) -> p c f", c=4),
                                T["Wuv"][0:512, :].rearrange(
                                    "(c p) f -> p c f", c=4))
            nc.gpsimd.dma_start(wqp_r[:].rearrange("p (c f) -> p c f", c=4),
                                T["Wqp"][:, :].rearrange(
                                    "(c p) f -> p c f", c=4))
            for hc in range(4):
                nc.gpsimd.dma_start(wos_r[hc][:],
                                    T["WoS"][hc * 128:(hc + 1) * 128, :])
            nc.gpsimd.dma_start(maskp[0][:], T["maskp0"][:])
            nc.gpsimd.dma_start(maskp[1][:], T["maskp1"][:])
            # preload exp table while phase A computes
            nc.scalar.activation(dume[:], dume[:], Ex)

            for g in range(3):
                ps = [psA.tile([128, TOK], F32, name=f"psl{g}{i}",
                               tag=f"ps{i}")
                      for i in range(4)]
                for mc in range(MC):
                    for i in range(4):
                        nc.tensor.matmul(
                            ps[i][:],
                            wdg[g][:, mc * 512 + i * 128:
                                   mc * 512 + (i + 1) * 128],
                            xat(mc),
                            start=(mc == 0), stop=(mc == MC - 1))
                for i in range(4):
                    lt = g * 4 + i
                    lat = latp.tile([128, TOK], BF16, name=f"lat{lt}",
                                    tag=f"lat{i}")
                    nc.scalar.activation(lat[:], ps[i][:], Ident,
                                         bias=bd[:, lt:lt + 1])
                    nc.sync.dma_start(
                        lat_cs[g][i * 128:(i + 1) * 128, :], lat[:])
                if g < 2:
                    nc.gpsimd.collective_compute(
                        "AllGather", mybir.AluOpType.bypass,
                        replica_groups=RG,
                        ins=[lat_cs[g][:]], outs=[lat_gs[g][:]])

            psk = psPK.tile([PHD, TOK], F32, name="pspk", tag="pspk")
            for mc in range(MC):
                nc.tensor.matmul(psk[:], wkp[:, mc * PHD:(mc + 1) * PHD],
                                 xat(mc),
                                 start=(mc == 0), stop=(mc == MC - 1))
            pkraw = evA.tile([PHD, TOK], BF16, name="pkraw", tag="pkraw")
            nc.scalar.activation(pkraw[:], psk[:], Ident, bias=bkp[:, 0:1])
            pk1 = evA.tile([PHD, TOK], BF16, name="pk1", tag="pk1")
            pku = evA.tile([PHD, TOK], BF16, name="pku", tag="pku")
            pkr = evA.tile([PHD, TOK], BF16, name="pkr", tag="pkr")
            nc.vector.tensor_mul(pk1[:], pkraw[:], cosk[:])
            nc.vector.tensor_mul(pku[:], pkraw[:], sink[:])
            nc.sync.dma_start(pkr[0:32, :], pku[32:64, :])
            nc.sync.dma_start(pkr[32:64, :], pku[0:32, :])
            nc.vector.tensor_add(pk1[:], pk1[:], pkr[:])
            nc.sync.dma_start(lat_cs[2][512:512 + PHD, :], pk1[:])
            nc.gpsimd.collective_compute(
                "AllGather", mybir.AluOpType.bypass, replica_groups=RG,
                ins=[lat_cs[2][:]], outs=[lat_gs[2][:]])

        # ---------------- Phase B1 ----------------
        with (
            tc.tile_pool(name="latg", bufs=1) as latgp,
            tc.tile_pool(name="wub", bufs=1) as wub,
            tc.tile_pool(name="ropet", bufs=1) as ropet,
            tc.tile_pool(name="psB", bufs=1, space="PSUM") as psB,
        ):
            wuq_r, wuk_r, wuv_r = [wuq0], [wuk0], [wuv0]
            for gg in (1, 2):
                rs = slice(gg * 512, (gg + 1) * 512)
                for wname, dst_list in (("Wuq", wuq_r), ("Wuk", wuk_r),
                                        ("Wuv", wuv_r)):
                    t = wub.tile([128, 2048], BF16, name=f"{wname}{gg}",
                                 tag=f"{wname}{gg}")
                    nc.gpsimd.dma_start(
                        t[:].rearrange("p (c f) -> p c f", c=4),
                        T[wname][rs, :].rearrange("(c p) f -> p c f", c=4))
                    dst_list.append(t)
            cosq = wub.tile([128, S], BF16, name="cosq", tag="cosq")
            sinq = wub.tile([128, S], BF16, name="sinq", tag="sinq")
            nc.gpsimd.dma_start(cosq[:], T["cosq"][:])
            nc.gpsimd.dma_start(sinq[:], T["sinq"][:])
            nc.gpsimd.dma_start(bk[:], T["bk"][:])
            nc.gpsimd.dma_start(bq[:], T["bq"][:])
            nc.gpsimd.dma_start(bqp[:], T["bqp"][:])
            nc.gpsimd.dma_start(bv[:], T["bv"][:])

            latg = []
            for lc in range(LC):
                g, r = divmod(lc, 4)
                t = latgp.tile([128, S], BF16, name=f"latg{lc}",
                               tag=f"lg{lc}")
                eng = nc.sync if lc % 2 == 0 else nc.scalar
                eng.dma_start(
                    t[:].rearrange("p (c t) -> p c t", c=4),
                    lat_gs[g][:, r * 128:(r + 1) * 128, :].rearrange(
                        "c p t -> p c t"))
                latg.append(t)
            nc.sync.dma_start(
                pk[:].rearrange("p (c t) -> p c t", c=4),
                lat_gs[2][:, 512:512 + PHD, :].rearrange("c p t -> p c t"))

            for c in range(4):
                cs = slice(c * 512, (c + 1) * 512)
                # wave 1: q (4 heads) + k (4 dim-chunks) -> 8 banks
                psq = [psB.tile([128, 512], F32, name=f"psq{c}{h}",
                               tag=f"b{h}") for h in range(4)]
                psk_ = [psB.tile([128, 512], F32, name=f"psk{c}{d}",
                                tag=f"b{4 + d}") for d in range(4)]
                for lc in range(LC):
                    g, i = divmod(lc, 4)
                    for h in range(4):
                        nc.tensor.matmul(
                            psq[h][:],
                            wuq_r[g][:, i * 512 + h * 128:
                                     i * 512 + (h + 1) * 128],
                            latg[lc][:, cs],
                            start=(lc == 0), stop=(lc == LC - 1))
                    for d in range(4):
                        nc.tensor.matmul(
                            psk_[d][:],
                            wuk_r[g][:, i * 512 + d * 128:
                                     i * 512 + (d + 1) * 128],
                            latg[lc][:, cs],
                            start=(lc == 0), stop=(lc == LC - 1))
                for h in range(4):
                    nc.scalar.activation(qt[h][:, cs], psq[h][:], Ident,
                                         bias=bq[:, h:h + 1])
                for d in range(4):
                    nc.scalar.activation(kct[d][:, cs], psk_[d][:], Ident,
                                         bias=bk[:, d:d + 1])
                # wave 2: v (4 token-tiles) + pos_q (2) -> 6 banks
                psv = [psB.tile([128, 512], F32, name=f"psv{c}{i}",
                               tag=f"b{i}") for i in range(4)]
                pspq = [psB.tile([128, 512], F32, name=f"pspq{c}{p_}",
                                tag=f"b{4 + p_}") for p_ in range(2)]
                for lc in range(LC):
                    g, i = divmod(lc, 4)
                    for j in range(4):
                        tcol = c * 512 + j * 128
                        nc.tensor.matmul(
                            psv[j][:], latg[lc][:, tcol:tcol + 128],
                            wuv_r[g][:, i * 512:(i + 1) * 512],
                            start=(lc == 0), stop=False)
                    if lc < LQC:
                        for p_ in range(2):
                            nc.tensor.matmul(
                                pspq[p_][:],
                                wqp_r[:, lc * 256 + p_ * 128:
                                      lc * 256 + (p_ + 1) * 128],
                                latg[lc][:, cs],
                                start=(lc == 0), stop=(lc == LQC - 1))
                for j in range(4):
                    nc.tensor.matmul(psv[j][:], ones1[:], bv[:],
                                     start=False, stop=True)
                    nc.scalar.copy(vt[c * 4 + j][:], psv[j][:])
                for p_ in range(2):
                    raw = ropet.tile([128, 512], BF16, name=f"pqr{c}{p_}",
                                     tag="praw")
                    nc.scalar.activation(raw[:], pspq[p_][:], Ident,
                                         bias=bqp[:, p_:p_ + 1])
                    t1 = ropet.tile([128, 512], BF16, name=f"t1{c}{p_}",
                                    tag="t1")
                    tu = ropet.tile([128, 512], BF16, name=f"tu{c}{p_}",
                                    tag="tu")
                    tr = ropet.tile([128, 512], BF16, name=f"tr{c}{p_}",
                                    tag="tr")
                    nc.vector.tensor_mul(t1[:], raw[:], cosq[:, cs])
                    nc.vector.tensor_mul(tu[:], raw[:], sinq[:, cs])
                    for h2 in range(2):
                        o = h2 * 64
                        nc.sync.dma_start(tr[o:o + 32, :],
                                          tu[o + 32:o + 64, :])
                        nc.sync.dma_start(tr[o + 32:o + 64, :],
                                          tu[o:o + 32, :])
                    nc.vector.tensor_add(t1[:], t1[:], tr[:])
                    nc.gpsimd.dma_start(pq[2 * p_][:, cs], t1[0:64, :])
                    nc.gpsimd.dma_start(pq[2 * p_ + 1][:, cs],
                                        t1[64:128, :])

        wug0_ctx.close()

        # ---------------- Phase B2 ----------------
        with (
            tc.tile_pool(name="ep", bufs=3) as ep,
            tc.tile_pool(name="e2p", bufs=2) as e2p,
            tc.tile_pool(name="rcp", bufs=2) as rcp,
            tc.tile_pool(name="oep", bufs=3) as oep,
            tc.tile_pool(name="pss", bufs=2, space="PSUM") as pss,
            tc.tile_pool(name="psav", bufs=2, space="PSUM") as psav,
            tc.tile_pool(name="psden", bufs=2, space="PSUM") as psden,
        ):
            for qB in range(4):
                qs = slice(qB * 512, (qB + 1) * 512)
                npair = 2 * qB + 2
                for h in range(4):
                    av = psav.tile([128, 512], F32, name=f"av{h}{qB}",
                                   tag="av")
                    den = psden.tile([128, 512], F32, name=f"den{h}{qB}",
                                     tag="den")
                    ee = []

                    def scores(p):
                        t = pss.tile([128, 1024], F32, name=f"s{h}{qB}{p}",
                                     tag="s")
                        for j in range(2):
                            kt = 2 * p + j
                            ks = slice(kt * 128, (kt + 1) * 128)
                            js = slice(j * 512, (j + 1) * 512)
                            nc.tensor.matmul(t[:, js], kct[h][:, ks],
                                             qt[h][:, qs],
                                             start=True, stop=False)
                            nc.tensor.matmul(t[:, js], pk[:, ks],
                                             pq[h][:, qs],
                                             start=False, stop=True)
                        e = ep.tile([128, 1024], BF16, name=f"e{h}{qB}{p}",
                                    tag="e")
                        nc.scalar.activation(e[:], t[:], Ex, scale=SCALE)
                        m = p - 2 * qB
                        if m >= 0:
                            e2 = e2p.tile([128, 1024], BF16,
                                          name=f"e2{h}{qB}{p}", tag="e2")
                            nc.vector.tensor_mul(e2[:], e[:], maskp[m][:])
                            e = e2
                        ee.append(e)

                    def accum(p):
                        e = ee[p]
                        for j in range(2):
                            kt = 2 * p + j
                            js = slice(j * 512, (j + 1) * 512)
                            st = (p == 0 and j == 0)
                            sto = (p == npair - 1 and j == 1)
                            nc.tensor.matmul(den[:], ONES[:], e[:, js],
                                             start=st, stop=sto)
                            nc.tensor.matmul(
                                av[:], vt[kt][:, h * 128:(h + 1) * 128],
                                e[:, js], start=st, stop=sto)

                    scores(0)
                    for p in range(1, npair):
                        scores(p)
                        accum(p - 1)
                    accum(npair - 1)
                    rc = rcp.tile([128, 512], F32, name=f"rc{h}{qB}",
                                  tag="rc")
                    nc.vector.reciprocal_approx_fast(rc[:], den[:])
                    nc.vector.tensor_mul(attn[h][:, qs], av[:], rc[:])

                # o_proj for this query block (borrows the av-pool banks)
                for mt in range(16):
                    op = psav.tile([128, 512], F32, name=f"op{qB}{mt}",
                                   tag="av")
                    for hc in range(4):
                        nc.tensor.matmul(
                            op[:],
                            wos_r[hc][:, mt * 128:(mt + 1) * 128],
                            attn[hc][:, qs],
                            start=(hc == 0), stop=(hc == 3))
                    oe = oep.tile([128, 512], BF16, name=f"oe{qB}{mt}",
                                  tag="oe")
                    nc.scalar.copy(oe[:], op[:])
                    eng = nc.gpsimd if mt % 2 == 0 else nc.sync
                    eng.dma_start(T["OT"][mt * 128:(mt + 1) * 128, qs],
                                  oe[:])

        persist_ctx.close()


def build_program():
    nc = bacc.Bacc("TRN2", target_bir_lowering=False, debug=False,
                   num_devices=NCORES)
    T = {}

    def inp(name, shape, dt=BF16):
        T[name] = nc.dram_tensor(name, shape, dt, kind="ExternalInput").ap()

    inp("xT", [MODEL, TOK])
    inp("Wd", [MODEL, L3])
    inp("Wuq", [L3, 512])
    inp("Wuk", [L3, 512])
    inp("Wuv", [L3, 512])
    inp("Wqp", [LATENT, 256])
    inp("Wkp", [MODEL, PHD])
    inp("WoS", [512, MODEL])
    inp("cosq", [128, S])
    inp("sinq", [128, S])
    inp("cosk", [PHD, TOK])
    inp("sink", [PHD, TOK])
    inp("bd", [128, LC], F32)
    inp("bk", [128, 4], F32)
    inp("bq", [128, 4], F32)
    inp("bqp", [128, 2], F32)
    inp("bkp", [PHD, 1], F32)
    inp("bv", [1, 512])
    inp("maskp0", [128, 1024])
    inp("maskp1", [128, 1024])
    T["OT"] = nc.dram_tensor("OT", [MODEL, S], BF16,
                             kind="ExternalOutput").ap()

    with tile.TileContext(nc) as tc:
        _emit(nc, tc, T)
    nc.compile()
    return nc


def host_inputs(inputs):
    import ml_dtypes
    bf16 = ml_dtypes.bfloat16
    x = np.ascontiguousarray(np.asarray(inputs["x"], np.float32))
    W_down = np.asarray(inputs["W_down"], np.float32)
    b_down = np.asarray(inputs["b_down"], np.float32)
    W_up = np.asarray(inputs["W_up"], np.float32)
    b_up = np.asarray(inputs["b_up"], np.float32)
    W_qpos = np.asarray(inputs["W_qpos"], np.float32)
    b_qpos = np.asarray(inputs["b_qpos"], np.float32)
    W_kpos = np.asarray(inputs["W_kpos"], np.float32)
    b_kpos = np.asarray(inputs["b_kpos"], np.float32)
    W_o = np.asarray(inputs["W_o"], np.float32)

    inv = (1.0 / ROPE_THETA ** (np.arange(0, PHD, 2, dtype=np.float32) / PHD))
    t_all = np.arange(S, dtype=np.float32)
    fr = np.outer(inv, t_all)                       # [32, S]
    cc = np.concatenate([np.cos(fr), np.cos(fr)], 0)        # [64, S]
    ss = np.sin(fr)
    ssn = np.concatenate([ss, -ss], 0)                      # [64, S]
    cosq = np.vstack([cc, cc]).astype(np.float32)           # [128, S]
    sinq = np.vstack([ssn, ssn]).astype(np.float32)

    qq = np.arange(512)[None, :]
    kk = np.arange(128)[:, None]
    masks = [np.where(qq >= kk + m * 128, 1.0, 0.0).astype(np.float32)
             for m in range(4)]
    maskp0 = np.ascontiguousarray(np.concatenate([masks[0], masks[1]], 1))
    maskp1 = np.ascontiguousarray(np.concatenate([masks[2], masks[3]], 1))

    common = {
        "Wd": np.ascontiguousarray(W_down),
        "Wkp": np.ascontiguousarray(W_kpos),
        "cosq": cosq, "sinq": sinq,
        "bd": np.ascontiguousarray(b_down.reshape(LC, 128).T),
        "bkp": np.ascontiguousarray(b_kpos[:, None]),
        "maskp0": maskp0, "maskp1": maskp1,
    }
    maps = []
    for c in range(NCORES):
        b, j = divmod(c, 4)
        ts = slice(j * TOK, (j + 1) * TOK)
        hs = slice(j * 512, (j + 1) * 512)
        m = dict(common)
        m["xT"] = np.ascontiguousarray(x[b, ts, :].T)
        m["Wuq"] = np.ascontiguousarray(W_up[:, :MODEL][:, hs])
        m["Wuk"] = np.ascontiguousarray(W_up[:, MODEL:2 * MODEL][:, hs])
        m["Wuv"] = np.ascontiguousarray(W_up[:, 2 * MODEL:][:, hs])
        m["Wqp"] = np.ascontiguousarray(W_qpos[:, j * 256:(j + 1) * 256])
        m["WoS"] = np.ascontiguousarray(W_o[hs, :])
        m["cosk"] = np.ascontiguousarray(cc[:, ts])
        m["sink"] = np.ascontiguousarray(ssn[:, ts])
        m["bq"] = np.ascontiguousarray(b_up[:MODEL][hs].reshape(4, 128).T)
        m["bk"] = np.ascontiguousarray(
            b_up[MODEL:2 * MODEL][hs].reshape(4, 128).T)
        m["bqp"] = np.ascontiguousarray(
            b_qpos[j * 256:(j + 1) * 256].reshape(2, 128).T)
        m["bv"] = np.ascontiguousarray(b_up[2 * MODEL:][hs][None, :])
        for key in list(m):
            if key not in F32_INPUTS:
                m[key] = np.ascontiguousarray(m[key]).astype(bf16)
        maps.append(m)
    return maps


_NC_CACHE = None


def _program():
    global _NC_CACHE
    if _NC_CACHE is None:
        _NC_CACHE = build_program()
    return _NC_CACHE


def kernel(**inputs) -> np.ndarray:
    nc = _program()
    maps = host_inputs(inputs)
    kwargs = {}
    if os.environ.get("BASSK_TRACE"):
        kwargs = dict(trace=True, trace_cores=list(range(NCORES)))
        td = os.environ.get("BASSK_TRACE_DIR")
        if td:
            kwargs["tmpdir"] = td
    res = bass_utils.run_bass_kernel_spmd(
        nc, maps, core_ids=list(range(NCORES)), **kwargs)
    kernel.last_results = res
    b_o = np.asarray(inputs["b_o"], np.float32)
    out = np.empty((B, S, MODEL), np.float32)
    for b in range(B):
        acc = np.asarray(res.results[b * 4]["OT"], np.float32)
        for c in range(b * 4 + 1, b * 4 + 4):
            acc = acc + np.asarray(res.results[c]["OT"], np.float32)
        out[b] = acc.T + b_o[None, :]
    return out


# revision 15
# speedup vs baseline: 1.1767x; 1.0263x over previous
"""MultiHeadLatentAttn TRN2 kernel (8 NeuronCores, uniform SPMD). v2.

Sharding:
  Phase A (token-parallel): core c (batch b=c//4, j=c%4) owns tokens
  [j*512,(j+1)*512) of batch b. Computes latentT [1536,512] + rope'd
  pos_kT [64,512]; AllGathers them within the batch group of 4 cores in
  3 pipelined chunks (g0/g1/g2+pos_k).
  Phase B1 (head-parallel): from the gathered latent each core computes
  qT, kT (its 4 heads only -- no k AllGather), pos_qT, v over ALL
  tokens, chunk-major so PSUM stays within 8 banks.
  Phase B2: causal attention for 4 heads, qB-outer; paired [128,1024]
  exp activations; fast-approx reciprocal; row-parallel o_proj per
  query block; bf16 partial output [2048 model, 2048 tokens].
  Host: sums the 4 partials per batch, transposes, adds b_o.
"""

import os
import sys

import numpy as np

for _p in ("/opt/trn_rl_repo", "/root/.axon_site/_ro/trn_rl_repo"):
    if os.path.isdir(_p) and _p not in sys.path:
        sys.path.append(_p)

import concourse.bass as bass
import concourse.mybir as mybir
import concourse.tile as tile
from concourse import bacc
from concourse import bass_utils

F32 = mybir.dt.float32
BF16 = mybir.dt.bfloat16

MODEL = 2048
LATENT = 512
L3 = 3 * LATENT            # 1536
NH = 16
HD = 128
PHD = 64
DC = HD + PHD              # 192
B, S = 2, 2048
TOK = 512                  # tokens per core in phase A
NCORES = 8
ROPE_THETA = 50000.0
SCALE = 1.0 / float(np.sqrt(DC))

MC = MODEL // 128          # 16
LC = L3 // 128             # 12
LQC = LATENT // 128        # 4

RG = [[0, 1, 2, 3], [4, 5, 6, 7]]
F32_INPUTS = {"bd", "bk", "bq", "bqp", "bkp"}


def _emit(nc, tc, T):
    from contextlib import ExitStack
    Ex = mybir.ActivationFunctionType.Exp
    Ident = mybir.ActivationFunctionType.Identity

    with tc.tile_pool(name="dram", bufs=1, space="DRAM") as dram:
        lat_cs = [dram.tile([512, TOK], BF16, name=f"lat_c{g}")
                  for g in range(2)]
        lat_cs.append(dram.tile([512 + PHD, TOK], BF16, name="lat_c2"))
        lat_gs = [dram.tile([4, 512, TOK], BF16, name=f"lat_g{g}")
                  for g in range(2)]
        lat_gs.append(dram.tile([4, 512 + PHD, TOK], BF16, name="lat_g2"))

        persist_ctx = ExitStack()
        persistp = persist_ctx.enter_context(
            tc.tile_pool(name="persist", bufs=1))
        qt = [persistp.tile([128, S], BF16, name=f"qt{h}", tag=f"qt{h}")
              for h in range(4)]
        kct = [persistp.tile([128, S], BF16, name=f"kct{h}", tag=f"kc{h}")
               for h in range(4)]
        vt = [persistp.tile([128, 512], BF16, name=f"vt{tt}", tag=f"vt{tt}")
              for tt in range(16)]
        pq = [persistp.tile([PHD, S], BF16, name=f"pq{h}", tag=f"pq{h}")
              for h in range(4)]
        pk = persistp.tile([PHD, S], BF16, name="pk", tag="pk")
        attn = [persistp.tile([128, S], BF16, name=f"attn{h}", tag=f"at{h}")
                for h in range(4)]
        wos_r = [persistp.tile([128, MODEL], BF16, name=f"wos{hc}",
                               tag=f"wos{hc}") for hc in range(4)]
        maskp = [persistp.tile([128, 1024], BF16, name=f"maskp{m}",
                               tag=f"mp{m}") for m in range(2)]
        bk = persistp.tile([128, 4], F32, name="bk", tag="bk")
        bq = persistp.tile([128, 4], F32, name="bq", tag="bq")
        bqp = persistp.tile([128, 2], F32, name="bqp", tag="bqp")
        bv = persistp.tile([1, 512], BF16, name="bv", tag="bv")
        ones1 = persistp.tile([1, 128], BF16, name="ones1", tag="ones1")
        ONES = persistp.tile([128, 128], BF16, name="ONES", tag="ONES")
        dume = persistp.tile([128, 1], BF16, name="dume", tag="dume")

        nc.vector.memset(ones1[:], 1.0)
        nc.vector.memset(ONES[:], 1.0)
        nc.vector.memset(dume[:], 0.0)

        # first lc-group of B1 weights: loaded during phase A, freed after B1
        wug0_ctx = ExitStack()
        wug0p = wug0_ctx.enter_context(tc.tile_pool(name="wug0", bufs=1))
        wuq0 = wug0p.tile([128, 2048], BF16, name="wuq0", tag="wuq0")
        wuk0 = wug0p.tile([128, 2048], BF16, name="wuk0", tag="wuk0")
        wuv0 = wug0p.tile([128, 2048], BF16, name="wuv0", tag="wuv0")
        wqp_r = wug0p.tile([128, 1024], BF16, name="wqp_r", tag="wqp")

        # ---------------- Phase A ----------------
        with (
            tc.tile_pool(name="xa", bufs=1) as xap,
            tc.tile_pool(name="wdp", bufs=1) as wdp,
            tc.tile_pool(name="latp", bufs=1) as latp,
            tc.tile_pool(name="cA", bufs=1) as cA,
            tc.tile_pool(name="evA", bufs=1) as evA,
            tc.tile_pool(name="psA", bufs=1, space="PSUM") as psA,
            tc.tile_pool(name="psPK", bufs=1, space="PSUM") as psPK,
        ):
            # x and Wd host-packed planar: plain contiguous 2D loads
            xa_all = xap.tile([128, MC * TOK], BF16, name="xa_all",
                              tag="xa")
            nc.sync.dma_start(xa_all[:], T["xT"][:])
            wdg = []
            for g in range(3):
                w = wdp.tile([128, MC * 512], BF16, name=f"wdg{g}",
                             tag=f"wdg{g}")
                nc.sync.dma_start(w[:], T["Wd"][:, g * 8192:(g + 1) * 8192])
                wdg.append(w)

            def xat(mc):
                return xa_all[:, mc * TOK:(mc + 1) * TOK]

            bd = cA.tile([128, LC], F32, name="bd", tag="bd")
            bkp = cA.tile([PHD, 1], F32, name="bkp", tag="bkp")
            wkp = cA.tile([128, MC * PHD], BF16, name="wkp", tag="wkp")
            cosk = cA.tile([PHD, TOK], BF16, name="cosk", tag="cosk")
            sink = cA.tile([PHD, TOK], BF16, name="sink", tag="sink")
            nc.sync.dma_start(bd[:], T["bd"][:])
            nc.sync.dma_start(bkp[:], T["bkp"][:])
            nc.sync.dma_start(wkp[:], T["Wkp"][:])
            nc.sync.dma_start(cosk[:], T["cosk"][:])
            nc.sync.dma_start(sink[:], T["sink"][:])

            # B1 first-group weights on the scalar HWDGE ring (host-packed
            # planar, cheap), before any ACT work exists.
            nc.scalar.dma_start(wuq0[:], T["Wuq"][:, 0:2048])
            nc.scalar.dma_start(wuk0[:], T["Wuk"][:, 0:2048])
            nc.scalar.dma_start(wuv0[:], T["Wuv"][:, 0:2048])
            nc.scalar.dma_start(wqp_r[:], T["Wqp"][:])
            # preload exp table while phase A computes
            nc.scalar.activation(dume[:], dume[:], Ex)

            for g in range(3):
                ps = [psA.tile([128, TOK], F32, name=f"psl{g}{i}",
                               tag=f"ps{i}")
                      for i in range(4)]
                for mc in range(MC):
                    for i in range(4):
                        nc.tensor.matmul(
                            ps[i][:],
                            wdg[g][:, mc * 512 + i * 128:
                                   mc * 512 + (i + 1) * 128],
                            xat(mc),
                            start=(mc == 0), stop=(mc == MC - 1))
                latG = latp.tile([128, 4 * TOK], BF16, name=f"latG{g}",
                                 tag="latG", bufs=2)
                for i in range(4):
                    lt = g * 4 + i
                    nc.scalar.activation(
                        latG[:, i * TOK:(i + 1) * TOK], ps[i][:], Ident,
                        bias=bd[:, lt:lt + 1])
                nc.sync.dma_start(
                    lat_cs[g][0:512, :].rearrange("(c p) t -> p c t", c=4),
                    latG[:].rearrange("p (c t) -> p c t", c=4))
                if g < 2:
                    nc.gpsimd.collective_compute(
                        "AllGather", mybir.AluOpType.bypass,
                        replica_groups=RG,
                        ins=[lat_cs[g][:]], outs=[lat_gs[g][:]])
                    # prefetch rides the gpsimd queue between AG triggers
                    if g == 0:
                        for hc in range(2):
                            nc.gpsimd.dma_start(
                                wos_r[hc][:],
                                T["WoS"][hc * 128:(hc + 1) * 128, :])
                    else:
                        for hc in range(2, 4):
                            nc.gpsimd.dma_start(
                                wos_r[hc][:],
                                T["WoS"][hc * 128:(hc + 1) * 128, :])
                        nc.gpsimd.dma_start(maskp[0][:], T["maskp0"][:])
                        nc.gpsimd.dma_start(maskp[1][:], T["maskp1"][:])

            psk = psPK.tile([PHD, TOK], F32, name="pspk", tag="pspk")
            for mc in range(MC):
                nc.tensor.matmul(psk[:], wkp[:, mc * PHD:(mc + 1) * PHD],
                                 xat(mc),
                                 start=(mc == 0), stop=(mc == MC - 1))
            pkraw = evA.tile([PHD, TOK], BF16, name="pkraw", tag="pkraw")
            nc.scalar.activation(pkraw[:], psk[:], Ident, bias=bkp[:, 0:1])
            pk1 = evA.tile([PHD, TOK], BF16, name="pk1", tag="pk1")
            pku = evA.tile([PHD, TOK], BF16, name="pku", tag="pku")
            pkr = evA.tile([PHD, TOK], BF16, name="pkr", tag="pkr")
            nc.vector.tensor_mul(pk1[:], pkraw[:], cosk[:])
            nc.vector.tensor_mul(pku[:], pkraw[:], sink[:])
            nc.sync.dma_start(pkr[0:32, :], pku[32:64, :])
            nc.sync.dma_start(pkr[32:64, :], pku[0:32, :])
            nc.vector.tensor_add(pk1[:], pk1[:], pkr[:])
            nc.sync.dma_start(lat_cs[2][512:512 + PHD, :], pk1[:])
            nc.gpsimd.collective_compute(
                "AllGather", mybir.AluOpType.bypass, replica_groups=RG,
                ins=[lat_cs[2][:]], outs=[lat_gs[2][:]])

        # ---------------- Phase B1 ----------------
        with (
            tc.tile_pool(name="latg", bufs=1) as latgp,
            tc.tile_pool(name="wub", bufs=1) as wub,
            tc.tile_pool(name="ropet", bufs=1) as ropet,
            tc.tile_pool(name="psB", bufs=1, space="PSUM") as psB,
        ):
            wuq_r, wuk_r, wuv_r = [wuq0], [wuk0], [wuv0]
            for gg in (1, 2):
                for wname, dst_list in (("Wuq", wuq_r), ("Wuk", wuk_r),
                                        ("Wuv", wuv_r)):
                    t = wub.tile([128, 2048], BF16, name=f"{wname}{gg}",
                                 tag=f"{wname}{gg}")
                    nc.gpsimd.dma_start(
                        t[:], T[wname][:, gg * 2048:(gg + 1) * 2048])
                    dst_list.append(t)
            cosq = wub.tile([128, S], BF16, name="cosq", tag="cosq")
            sinq = wub.tile([128, S], BF16, name="sinq", tag="sinq")
            nc.gpsimd.dma_start(cosq[:], T["cosq"][:])
            nc.gpsimd.dma_start(sinq[:], T["sinq"][:])
            nc.gpsimd.dma_start(bk[:], T["bk"][:])
            nc.gpsimd.dma_start(bq[:], T["bq"][:])
            nc.gpsimd.dma_start(bqp[:], T["bqp"][:])
            nc.gpsimd.dma_start(bv[:], T["bv"][:])

            latg = []
            for lc in range(LC):
                g, r = divmod(lc, 4)
                t = latgp.tile([128, S], BF16, name=f"latg{lc}",
                               tag=f"lg{lc}")
                eng = nc.sync if lc % 2 == 0 else nc.scalar
                eng.dma_start(
                    t[:].rearrange("p (c t) -> p c t", c=4),
                    lat_gs[g][:, r * 128:(r + 1) * 128, :].rearrange(
                        "c p t -> p c t"))
                latg.append(t)
            nc.sync.dma_start(
                pk[:].rearrange("p (c t) -> p c t", c=4),
                lat_gs[2][:, 512:512 + PHD, :].rearrange("c p t -> p c t"))

            for c in range(4):
                cs = slice(c * 512, (c + 1) * 512)
                # wave 1: q (4 heads) + k (4 dim-chunks) -> 8 banks
                psq = [psB.tile([128, 512], F32, name=f"psq{c}{h}",
                               tag=f"b{h}") for h in range(4)]
                psk_ = [psB.tile([128, 512], F32, name=f"psk{c}{d}",
                                tag=f"b{4 + d}") for d in range(4)]
                for lc in range(LC):
                    g, i = divmod(lc, 4)
                    for h in range(4):
                        nc.tensor.matmul(
                            psq[h][:],
                            wuq_r[g][:, i * 512 + h * 128:
                                     i * 512 + (h + 1) * 128],
                            latg[lc][:, cs],
                            start=(lc == 0), stop=(lc == LC - 1))
                    for d in range(4):
                        nc.tensor.matmul(
                            psk_[d][:],
                            wuk_r[g][:, i * 512 + d * 128:
                                     i * 512 + (d + 1) * 128],
                            latg[lc][:, cs],
                            start=(lc == 0), stop=(lc == LC - 1))
                for h in range(4):
                    nc.scalar.activation(qt[h][:, cs], psq[h][:], Ident,
                                         bias=bq[:, h:h + 1])
                for d in range(4):
                    nc.scalar.activation(kct[d][:, cs], psk_[d][:], Ident,
                                         bias=bk[:, d:d + 1])
                # wave 2: v (4 token-tiles) + pos_q (2) -> 6 banks
                psv = [psB.tile([128, 512], F32, name=f"psv{c}{i}",
                               tag=f"b{i}") for i in range(4)]
                pspq = [psB.tile([128, 512], F32, name=f"pspq{c}{p_}",
                                tag=f"b{4 + p_}") for p_ in range(2)]
                for lc in range(LC):
                    g, i = divmod(lc, 4)
                    for j in range(4):
                        tcol = c * 512 + j * 128
                        nc.tensor.matmul(
                            psv[j][:], latg[lc][:, tcol:tcol + 128],
                            wuv_r[g][:, i * 512:(i + 1) * 512],
                            start=(lc == 0), stop=False)
                    if lc < LQC:
                        for p_ in range(2):
                            nc.tensor.matmul(
                                pspq[p_][:],
                                wqp_r[:, lc * 256 + p_ * 128:
                                      lc * 256 + (p_ + 1) * 128],
                                latg[lc][:, cs],
                                start=(lc == 0), stop=(lc == LQC - 1))
                for j in range(4):
                    nc.tensor.matmul(psv[j][:], ones1[:], bv[:],
                                     start=False, stop=True)
                    nc.scalar.copy(vt[c * 4 + j][:], psv[j][:])
                for p_ in range(2):
                    raw = ropet.tile([128, 512], BF16, name=f"pqr{c}{p_}",
                                     tag="praw")
                    nc.scalar.activation(raw[:], pspq[p_][:], Ident,
                                         bias=bqp[:, p_:p_ + 1])
                    t1 = ropet.tile([128, 512], BF16, name=f"t1{c}{p_}",
                                    tag="t1")
                    tu = ropet.tile([128, 512], BF16, name=f"tu{c}{p_}",
                                    tag="tu")
                    tr = ropet.tile([128, 512], BF16, name=f"tr{c}{p_}",
                                    tag="tr")
                    nc.vector.tensor_mul(t1[:], raw[:], cosq[:, cs])
                    nc.vector.tensor_mul(tu[:], raw[:], sinq[:, cs])
                    for h2 in range(2):
                        o = h2 * 64
                        nc.sync.dma_start(tr[o:o + 32, :],
                                          tu[o + 32:o + 64, :])
                        nc.sync.dma_start(tr[o + 32:o + 64, :],
                                          tu[o:o + 32, :])
                    nc.vector.tensor_add(t1[:], t1[:], tr[:])
                    nc.gpsimd.dma_start(pq[2 * p_][:, cs], t1[0:64, :])
                    nc.gpsimd.dma_start(pq[2 * p_ + 1][:, cs],
                                        t1[64:128, :])

        wug0_ctx.close()

        # ---------------- Phase B2 ----------------
        with (
            tc.tile_pool(name="ep", bufs=4) as ep,
            tc.tile_pool(name="e2p", bufs=2) as e2p,
            tc.tile_pool(name="rcp", bufs=2) as rcp,
            tc.tile_pool(name="oep", bufs=3) as oep,
            tc.tile_pool(name="pss", bufs=3, space="PSUM") as pss,
            tc.tile_pool(name="psav", bufs=1, space="PSUM") as psav,
            tc.tile_pool(name="psden", bufs=1, space="PSUM") as psden,
        ):
            for qB in range(4):
                qs = slice(qB * 512, (qB + 1) * 512)
                npair = 2 * qB + 2
                for h in range(4):
                    av = psav.tile([128, 512], F32, name=f"av{h}{qB}",
                                   tag="av")
                    den = psden.tile([128, 512], F32, name=f"den{h}{qB}",
                                     tag="den")
                    ee = []

                    def scores(p):
                        t = pss.tile([128, 1024], F32, name=f"s{h}{qB}{p}",
                                     tag="s")
                        for j in range(2):
                            kt = 2 * p + j
                            ks = slice(kt * 128, (kt + 1) * 128)
                            js = slice(j * 512, (j + 1) * 512)
                            nc.tensor.matmul(t[:, js], kct[h][:, ks],
                                             qt[h][:, qs],
                                             start=True, stop=False)
                            nc.tensor.matmul(t[:, js], pk[:, ks],
                                             pq[h][:, qs],
                                             start=False, stop=True)
                        e = ep.tile([128, 1024], BF16, name=f"e{h}{qB}{p}",
                                    tag="e")
                        nc.scalar.activation(e[:], t[:], Ex, scale=SCALE)
                        m = p - 2 * qB
                        if m >= 0:
                            e2 = e2p.tile([128, 1024], BF16,
                                          name=f"e2{h}{qB}{p}", tag="e2")
                            nc.vector.tensor_mul(e2[:], e[:], maskp[m][:])
                            e = e2
                        ee.append(e)

                    def accum(p):
                        e = ee[p]
                        for j in range(2):
                            kt = 2 * p + j
                            js = slice(j * 512, (j + 1) * 512)
                            st = (p == 0 and j == 0)
                            sto = (p == npair - 1 and j == 1)
                            nc.tensor.matmul(den[:], ONES[:], e[:, js],
                                             start=st, stop=sto)
                            nc.tensor.matmul(
                                av[:], vt[kt][:, h * 128:(h + 1) * 128],
                                e[:, js], start=st, stop=sto)

                    # run scores two pairs ahead of accumulation to hide
                    # cross-engine semaphore latency
                    scores(0)
                    if npair > 1:
                        scores(1)
                    for p in range(2, npair):
                        scores(p)
                        accum(p - 2)
                    accum(npair - 2) if npair > 1 else None
                    accum(npair - 1)
                    rc = rcp.tile([128, 512], F32, name=f"rc{h}{qB}",
                                  tag="rc")
                    nc.vector.reciprocal_approx_fast(rc[:], den[:])
                    nc.vector.tensor_mul(attn[h][:, qs], av[:], rc[:])

                # o_proj for this query block (borrows av+den banks)
                for mt in range(16):
                    pool = psav if mt % 2 == 0 else psden
                    tg = "av" if mt % 2 == 0 else "den"
                    op = pool.tile([128, 512], F32, name=f"op{qB}{mt}",
                                   tag=tg)
                    for hc in range(4):
                        nc.tensor.matmul(
                            op[:],
                            wos_r[hc][:, mt * 128:(mt + 1) * 128],
                            attn[hc][:, qs],
                            start=(hc == 0), stop=(hc == 3))
                    oe = oep.tile([128, 512], BF16, name=f"oe{qB}{mt}",
                                  tag="oe")
                    nc.scalar.copy(oe[:], op[:])
                    eng = nc.gpsimd if mt % 2 == 0 else nc.sync
                    eng.dma_start(T["OT"][mt * 128:(mt + 1) * 128, qs],
                                  oe[:])

        persist_ctx.close()


def build_program():
    nc = bacc.Bacc("TRN2", target_bir_lowering=False, debug=False,
                   num_devices=NCORES)
    T = {}

    def inp(name, shape, dt=BF16):
        T[name] = nc.dram_tensor(name, shape, dt, kind="ExternalInput").ap()

    inp("xT", [128, MC * TOK])
    inp("Wd", [128, 3 * MC * 512])
    inp("Wuq", [128, 3 * 2048])
    inp("Wuk", [128, 3 * 2048])
    inp("Wuv", [128, 3 * 2048])
    inp("Wqp", [128, 1024])
    inp("Wkp", [128, MC * PHD])
    inp("WoS", [512, MODEL])
    inp("cosq", [128, S])
    inp("sinq", [128, S])
    inp("cosk", [PHD, TOK])
    inp("sink", [PHD, TOK])
    inp("bd", [128, LC], F32)
    inp("bk", [128, 4], F32)
    inp("bq", [128, 4], F32)
    inp("bqp", [128, 2], F32)
    inp("bkp", [PHD, 1], F32)
    inp("bv", [1, 512])
    inp("maskp0", [128, 1024])
    inp("maskp1", [128, 1024])
    T["OT"] = nc.dram_tensor("OT", [MODEL, S], BF16,
                             kind="ExternalOutput").ap()

    with tile.TileContext(nc) as tc:
        _emit(nc, tc, T)
    nc.compile()
    return nc


def host_inputs(inputs):
    import ml_dtypes
    bf16 = ml_dtypes.bfloat16
    x = np.ascontiguousarray(np.asarray(inputs["x"], np.float32))
    W_down = np.asarray(inputs["W_down"], np.float32)
    b_down = np.asarray(inputs["b_down"], np.float32)
    W_up = np.asarray(inputs["W_up"], np.float32)
    b_up = np.asarray(inputs["b_up"], np.float32)
    W_qpos = np.asarray(inputs["W_qpos"], np.float32)
    b_qpos = np.asarray(inputs["b_qpos"], np.float32)
    W_kpos = np.asarray(inputs["W_kpos"], np.float32)
    b_kpos = np.asarray(inputs["b_kpos"], np.float32)
    W_o = np.asarray(inputs["W_o"], np.float32)

    inv = (1.0 / ROPE_THETA ** (np.arange(0, PHD, 2, dtype=np.float32) / PHD))
    t_all = np.arange(S, dtype=np.float32)
    fr = np.outer(inv, t_all)                       # [32, S]
    cc = np.concatenate([np.cos(fr), np.cos(fr)], 0)        # [64, S]
    ss = np.sin(fr)
    ssn = np.concatenate([ss, -ss], 0)                      # [64, S]
    cosq = np.vstack([cc, cc]).astype(np.float32)           # [128, S]
    sinq = np.vstack([ssn, ssn]).astype(np.float32)

    qq = np.arange(512)[None, :]
    kk = np.arange(128)[:, None]
    masks = [np.where(qq >= kk + m * 128, 1.0, 0.0).astype(np.float32)
             for m in range(4)]
    maskp0 = np.ascontiguousarray(np.concatenate([masks[0], masks[1]], 1))
    maskp1 = np.ascontiguousarray(np.concatenate([masks[2], masks[3]], 1))

    def pack_rows(w, nchunk):
        # [nchunk*128, F] -> [128, nchunk*F] with chunk-major columns
        F = w.shape[1]
        return np.ascontiguousarray(
            w.reshape(nchunk, 128, F).transpose(1, 0, 2).reshape(
                128, nchunk * F))

    def pack_wu(w):
        # [1536, 512] -> [128, 3*2048]: cols gg*2048 + i*512 + f
        return np.ascontiguousarray(
            w.reshape(3, 4, 128, 512).transpose(2, 0, 1, 3).reshape(
                128, 6144))

    common = {
        "Wd": np.ascontiguousarray(
            W_down.reshape(MC, 128, 3, 512).transpose(1, 2, 0, 3).reshape(
                128, 3 * MC * 512)),
        "Wkp": pack_rows(W_kpos, MC),
        "cosq": cosq, "sinq": sinq,
        "bd": np.ascontiguousarray(b_down.reshape(LC, 128).T),
        "bkp": np.ascontiguousarray(b_kpos[:, None]),
        "maskp0": maskp0, "maskp1": maskp1,
    }
    maps = []
    for c in range(NCORES):
        b, j = divmod(c, 4)
        ts = slice(j * TOK, (j + 1) * TOK)
        hs = slice(j * 512, (j + 1) * 512)
        m = dict(common)
        m["xT"] = pack_rows(np.ascontiguousarray(x[b, ts, :].T), MC)
        m["Wuq"] = pack_wu(W_up[:, :MODEL][:, hs])
        m["Wuk"] = pack_wu(W_up[:, MODEL:2 * MODEL][:, hs])
        m["Wuv"] = pack_wu(W_up[:, 2 * MODEL:][:, hs])
        m["Wqp"] = pack_rows(
            np.ascontiguousarray(W_qpos[:, j * 256:(j + 1) * 256]), 4)
        m["WoS"] = np.ascontiguousarray(W_o[hs, :])
        m["cosk"] = np.ascontiguousarray(cc[:, ts])
        m["sink"] = np.ascontiguousarray(ssn[:, ts])
        m["bq"] = np.ascontiguousarray(b_up[:MODEL][hs].reshape(4, 128).T)
        m["bk"] = np.ascontiguousarray(
            b_up[MODEL:2 * MODEL][hs].reshape(4, 128).T)
        m["bqp"] = np.ascontiguousarray(
            b_qpos[j * 256:(j + 1) * 256].reshape(2, 128).T)
        m["bv"] = np.ascontiguousarray(b_up[2 * MODEL:][hs][None, :])
        for key in list(m):
            if key not in F32_INPUTS:
                m[key] = np.ascontiguousarray(m[key]).astype(bf16)
        maps.append(m)
    return maps


_NC_CACHE = None


def _program():
    global _NC_CACHE
    if _NC_CACHE is None:
        _NC_CACHE = build_program()
    return _NC_CACHE


def kernel(**inputs) -> np.ndarray:
    nc = _program()
    maps = host_inputs(inputs)
    kwargs = {}
    if os.environ.get("BASSK_TRACE"):
        kwargs = dict(trace=True, trace_cores=list(range(NCORES)))
        td = os.environ.get("BASSK_TRACE_DIR")
        if td:
            kwargs["tmpdir"] = td
    res = bass_utils.run_bass_kernel_spmd(
        nc, maps, core_ids=list(range(NCORES)), **kwargs)
    kernel.last_results = res
    b_o = np.asarray(inputs["b_o"], np.float32)
    out = np.empty((B, S, MODEL), np.float32)
    for b in range(B):
        acc = np.asarray(res.results[b * 4]["OT"], np.float32)
        for c in range(b * 4 + 1, b * 4 + 4):
            acc = acc + np.asarray(res.results[c]["OT"], np.float32)
        out[b] = acc.T + b_o[None, :]
    return out


# revision 23
# speedup vs baseline: 1.1859x; 1.0078x over previous
"""MultiHeadLatentAttn TRN2 kernel (8 NeuronCores, uniform SPMD). v2.

Sharding:
  Phase A (token-parallel): core c (batch b=c//4, j=c%4) owns tokens
  [j*512,(j+1)*512) of batch b. Computes latentT [1536,512] + rope'd
  pos_kT [64,512]; AllGathers them within the batch group of 4 cores in
  3 pipelined chunks (g0/g1/g2+pos_k).
  Phase B1 (head-parallel): from the gathered latent each core computes
  qT, kT (its 4 heads only -- no k AllGather), pos_qT, v over ALL
  tokens, chunk-major so PSUM stays within 8 banks.
  Phase B2: causal attention for 4 heads, qB-outer; paired [128,1024]
  exp activations; fast-approx reciprocal; row-parallel o_proj per
  query block; bf16 partial output [2048 model, 2048 tokens].
  Host: sums the 4 partials per batch, transposes, adds b_o.
"""

import os
import sys

import numpy as np

for _p in ("/opt/trn_rl_repo", "/root/.axon_site/_ro/trn_rl_repo"):
    if os.path.isdir(_p) and _p not in sys.path:
        sys.path.append(_p)

import concourse.bass as bass
import concourse.mybir as mybir
import concourse.tile as tile
from concourse import bacc
from concourse import bass_utils

F32 = mybir.dt.float32
BF16 = mybir.dt.bfloat16
FP8 = mybir.dt.float8e4
DR = mybir.MatmulPerfMode.DoubleRow

MODEL = 2048
LATENT = 512
L3 = 3 * LATENT            # 1536
NH = 16
HD = 128
PHD = 64
DC = HD + PHD              # 192
B, S = 2, 2048
TOK = 512                  # tokens per core in phase A
NCORES = 8
ROPE_THETA = 50000.0
SCALE = 1.0 / float(np.sqrt(DC))

MC = MODEL // 128          # 16
LC = L3 // 128             # 12
LQC = LATENT // 128        # 4

RG = [[0, 1, 2, 3], [4, 5, 6, 7]]
F32_INPUTS = {"bd", "bk", "bq", "bqp", "bkp"}


def _emit(nc, tc, T):
    from contextlib import ExitStack
    Ex = mybir.ActivationFunctionType.Exp
    Ident = mybir.ActivationFunctionType.Identity

    with tc.tile_pool(name="dram", bufs=1, space="DRAM") as dram:
        lat_cs = [dram.tile([512, TOK], BF16, name=f"lat_c{g}")
                  for g in range(2)]
        lat_cs.append(dram.tile([512 + PHD, TOK], BF16, name="lat_c2"))
        lat_gs = [dram.tile([4, 512, TOK], BF16, name=f"lat_g{g}")
                  for g in range(2)]
        lat_gs.append(dram.tile([4, 512 + PHD, TOK], BF16, name="lat_g2"))
        warm_in = dram.tile([1, 128], BF16, name="warm_in")
        warm_out = dram.tile([4, 1, 128], BF16, name="warm_out")
        # fire a tiny AllGather immediately: pulls the CC entry barrier and
        # ring warm-up to t~0 instead of serializing before the first real AG
        nc.gpsimd.collective_compute(
            "AllGather", mybir.AluOpType.bypass, replica_groups=RG,
            ins=[warm_in[:]], outs=[warm_out[:]])

        persist_ctx = ExitStack()
        persistp = persist_ctx.enter_context(
            tc.tile_pool(name="persist", bufs=1))
        qt = [persistp.tile([128, S], BF16, name=f"qt{h}", tag=f"qt{h}")
              for h in range(4)]
        kct = [persistp.tile([128, S], BF16, name=f"kct{h}", tag=f"kc{h}")
               for h in range(4)]
        vt = [persistp.tile([128, 512], BF16, name=f"vt{tt}", tag=f"vt{tt}")
              for tt in range(16)]
        pq = [persistp.tile([PHD, S], BF16, name=f"pq{h}", tag=f"pq{h}")
              for h in range(4)]
        pk = persistp.tile([PHD, S], BF16, name="pk", tag="pk")
        attn = [persistp.tile([128, S], BF16, name=f"attn{h}", tag=f"at{h}")
                for h in range(4)]
        wos_r = [persistp.tile([128, MODEL], BF16, name=f"wos{hc}",
                               tag=f"wos{hc}") for hc in range(4)]
        maskp = [persistp.tile([128, 1024], BF16, name=f"maskp{m}",
                               tag=f"mp{m}") for m in range(2)]
        bk = persistp.tile([128, 4], F32, name="bk", tag="bk")
        bq = persistp.tile([128, 4], F32, name="bq", tag="bq")
        bqp = persistp.tile([128, 2], F32, name="bqp", tag="bqp")
        bv = persistp.tile([1, 512], BF16, name="bv", tag="bv")
        ones1 = persistp.tile([1, 128], BF16, name="ones1", tag="ones1")
        ONES = persistp.tile([128, 128], BF16, name="ONES", tag="ONES")
        ONES8 = persistp.tile([128, 128], FP8, name="ONES8", tag="ONES8")
        nc.vector.memset(ONES8[:], 1.0)
        dume = persistp.tile([128, 1], BF16, name="dume", tag="dume")

        nc.vector.memset(ones1[:], 1.0)
        nc.vector.memset(ONES[:], 1.0)
        nc.vector.memset(dume[:], 0.0)

        # first lc-group of B1 weights: loaded during phase A, freed after B1
        wug0_ctx = ExitStack()
        wug0p = wug0_ctx.enter_context(tc.tile_pool(name="wug0", bufs=1))
        wuq0 = wug0p.tile([128, 2048], BF16, name="wuq0", tag="wuq0")
        wuk0 = wug0p.tile([128, 2048], BF16, name="wuk0", tag="wuk0")
        wuv0 = wug0p.tile([128, 2048], BF16, name="wuv0", tag="wuv0")
        wqp_r = wug0p.tile([128, 1024], BF16, name="wqp_r", tag="wqp")

        # ---------------- Phase A ----------------
        with (
            tc.tile_pool(name="xa", bufs=1) as xap,
            tc.tile_pool(name="wdp", bufs=1) as wdp,
            tc.tile_pool(name="latp", bufs=1) as latp,
            tc.tile_pool(name="cA", bufs=1) as cA,
            tc.tile_pool(name="evA", bufs=1) as evA,
            tc.tile_pool(name="psA", bufs=1, space="PSUM") as psA,
            tc.tile_pool(name="psPK", bufs=1, space="PSUM") as psPK,
        ):
            # x and Wd host-packed planar, split so the first matmuls
            # unblock in ~10us; wd groups 1/2 ride the scalar ring.
            xq = [xap.tile([128, 4 * TOK], BF16, name=f"xq{i}",
                           tag=f"xq{i}") for i in range(4)]
            wda_ = [wdp.tile([128, 8 * 512], BF16, name=f"wda{g}",
                             tag=f"wda{g}") for g in range(3)]
            wdb_ = [wdp.tile([128, 8 * 512], BF16, name=f"wdb{g}",
                             tag=f"wdb{g}") for g in range(3)]
            nc.sync.dma_start(xq[0][:], T["xT"][:, 0:2048])
            nc.sync.dma_start(xq[1][:], T["xT"][:, 2048:4096])
            nc.sync.dma_start(wda_[0][:], T["Wd"][:, 0:4096])
            nc.sync.dma_start(xq[2][:], T["xT"][:, 4096:6144])
            nc.sync.dma_start(xq[3][:], T["xT"][:, 6144:8192])
            nc.sync.dma_start(wdb_[0][:], T["Wd"][:, 4096:8192])
            nc.scalar.dma_start(wda_[1][:], T["Wd"][:, 8192:12288])
            nc.scalar.dma_start(wdb_[1][:], T["Wd"][:, 12288:16384])
            nc.scalar.dma_start(wda_[2][:], T["Wd"][:, 16384:20480])
            nc.scalar.dma_start(wdb_[2][:], T["Wd"][:, 20480:24576])

            def xat(mc):
                return xq[mc // 4][:, (mc % 4) * TOK:(mc % 4 + 1) * TOK]

            def wdt(g, mc, i):
                w = wda_[g] if mc < 8 else wdb_[g]
                o = (mc % 8) * 512 + i * 128
                return w[:, o:o + 128]

            bd = cA.tile([128, LC], F32, name="bd", tag="bd")
            bkp = cA.tile([PHD, 1], F32, name="bkp", tag="bkp")
            wkp = cA.tile([128, MC * PHD], BF16, name="wkp", tag="wkp")
            cosk = cA.tile([PHD, TOK], BF16, name="cosk", tag="cosk")
            sink = cA.tile([PHD, TOK], BF16, name="sink", tag="sink")
            nc.sync.dma_start(bd[:], T["bd"][:])
            nc.sync.dma_start(bkp[:], T["bkp"][:])
            nc.sync.dma_start(wkp[:], T["Wkp"][:])
            nc.sync.dma_start(cosk[:], T["cosk"][:])
            nc.sync.dma_start(sink[:], T["sink"][:])

            # B1 first-group weights on the scalar HWDGE ring (host-packed
            # planar, cheap), before any ACT work exists.
            nc.scalar.dma_start(wuq0[:], T["Wuq"][:, 0:2048])
            nc.scalar.dma_start(wuk0[:], T["Wuk"][:, 0:2048])
            nc.scalar.dma_start(wuv0[:], T["Wuv"][:, 0:2048])
            nc.scalar.dma_start(wqp_r[:], T["Wqp"][:])
            # preload exp table while phase A computes
            nc.scalar.activation(dume[:], dume[:], Ex)

            for g in range(3):
                ps = [psA.tile([128, TOK], F32, name=f"psl{g}{i}",
                               tag=f"ps{i}")
                      for i in range(4)]
                for mc in range(MC):
                    for i in range(4):
                        nc.tensor.matmul(
                            ps[i][:], wdt(g, mc, i), xat(mc),
                            start=(mc == 0), stop=(mc == MC - 1))
                latG = latp.tile([128, 4 * TOK], BF16, name=f"latG{g}",
                                 tag="latG", bufs=2)
                for i in range(4):
                    lt = g * 4 + i
                    nc.scalar.activation(
                        latG[:, i * TOK:(i + 1) * TOK], ps[i][:], Ident,
                        bias=bd[:, lt:lt + 1])
                nc.sync.dma_start(
                    lat_cs[g][0:512, :].rearrange("(c p) t -> p c t", c=4),
                    latG[:].rearrange("p (c t) -> p c t", c=4))
                if g < 2:
                    nc.gpsimd.collective_compute(
                        "AllGather", mybir.AluOpType.bypass,
                        replica_groups=RG,
                        ins=[lat_cs[g][:]], outs=[lat_gs[g][:]])
                    # prefetch rides the gpsimd queue between AG triggers
                    if g == 0:
                        for hc in range(2):
                            nc.gpsimd.dma_start(
                                wos_r[hc][:],
                                T["WoS"][hc * 128:(hc + 1) * 128, :])
                    else:
                        for hc in range(2, 4):
                            nc.gpsimd.dma_start(
                                wos_r[hc][:],
                                T["WoS"][hc * 128:(hc + 1) * 128, :])
                        nc.gpsimd.dma_start(maskp[0][:], T["maskp0"][:])
                        nc.gpsimd.dma_start(maskp[1][:], T["maskp1"][:])

            psk = psPK.tile([PHD, TOK], F32, name="pspk", tag="pspk")
            for mc in range(MC):
                nc.tensor.matmul(psk[:], wkp[:, mc * PHD:(mc + 1) * PHD],
                                 xat(mc),
                                 start=(mc == 0), stop=(mc == MC - 1))
            pkraw = evA.tile([PHD, TOK], BF16, name="pkraw", tag="pkraw")
            nc.scalar.activation(pkraw[:], psk[:], Ident, bias=bkp[:, 0:1])
            pk1 = evA.tile([PHD, TOK], BF16, name="pk1", tag="pk1")
            pku = evA.tile([PHD, TOK], BF16, name="pku", tag="pku")
            pkr = evA.tile([PHD, TOK], BF16, name="pkr", tag="pkr")
            nc.vector.tensor_mul(pk1[:], pkraw[:], cosk[:])
            nc.vector.tensor_mul(pku[:], pkraw[:], sink[:])
            nc.sync.dma_start(pkr[0:32, :], pku[32:64, :])
            nc.sync.dma_start(pkr[32:64, :], pku[0:32, :])
            nc.vector.tensor_add(pk1[:], pk1[:], pkr[:])
            nc.sync.dma_start(lat_cs[2][512:512 + PHD, :], pk1[:])
            nc.gpsimd.collective_compute(
                "AllGather", mybir.AluOpType.bypass, replica_groups=RG,
                ins=[lat_cs[2][:]], outs=[lat_gs[2][:]])

        # ---------------- Phase B1 ----------------
        with (
            tc.tile_pool(name="latg", bufs=1) as latgp,
            tc.tile_pool(name="wub", bufs=1) as wub,
            tc.tile_pool(name="ropet", bufs=1) as ropet,
            tc.tile_pool(name="psB", bufs=1, space="PSUM") as psB,
        ):
            wuq_r, wuk_r, wuv_r = [wuq0], [wuk0], [wuv0]
            for gg in (1, 2):
                for wname, dst_list in (("Wuq", wuq_r), ("Wuk", wuk_r),
                                        ("Wuv", wuv_r)):
                    t = wub.tile([128, 2048], BF16, name=f"{wname}{gg}",
                                 tag=f"{wname}{gg}")
                    nc.gpsimd.dma_start(
                        t[:], T[wname][:, gg * 2048:(gg + 1) * 2048])
                    dst_list.append(t)
            cosq = wub.tile([128, S], BF16, name="cosq", tag="cosq")
            sinq = wub.tile([128, S], BF16, name="sinq", tag="sinq")
            nc.gpsimd.dma_start(cosq[:], T["cosq"][:])
            nc.gpsimd.dma_start(sinq[:], T["sinq"][:])
            nc.gpsimd.dma_start(bk[:], T["bk"][:])
            nc.gpsimd.dma_start(bq[:], T["bq"][:])
            nc.gpsimd.dma_start(bqp[:], T["bqp"][:])
            nc.gpsimd.dma_start(bv[:], T["bv"][:])

            latg = []
            for lc in range(LC):
                g, r = divmod(lc, 4)
                t = latgp.tile([128, S], BF16, name=f"latg{lc}",
                               tag=f"lg{lc}")
                eng = nc.sync if lc % 2 == 0 else nc.scalar
                eng.dma_start(
                    t[:].rearrange("p (c t) -> p c t", c=4),
                    lat_gs[g][:, r * 128:(r + 1) * 128, :].rearrange(
                        "c p t -> p c t"))
                latg.append(t)
            nc.sync.dma_start(
                pk[:].rearrange("p (c t) -> p c t", c=4),
                lat_gs[2][:, 512:512 + PHD, :].rearrange("c p t -> p c t"))

            for c in range(4):
                cs = slice(c * 512, (c + 1) * 512)
                # wave 1: q (4 heads) + k (4 dim-chunks) -> 8 banks
                psq = [psB.tile([128, 512], F32, name=f"psq{c}{h}",
                               tag=f"b{h}") for h in range(4)]
                psk_ = [psB.tile([128, 512], F32, name=f"psk{c}{d}",
                                tag=f"b{4 + d}") for d in range(4)]
                for lc in range(LC):
                    g, i = divmod(lc, 4)
                    for h in range(4):
                        nc.tensor.matmul(
                            psq[h][:],
                            wuq_r[g][:, i * 512 + h * 128:
                                     i * 512 + (h + 1) * 128],
                            latg[lc][:, cs],
                            start=(lc == 0), stop=(lc == LC - 1))
                    for d in range(4):
                        nc.tensor.matmul(
                            psk_[d][:],
                            wuk_r[g][:, i * 512 + d * 128:
                                     i * 512 + (d + 1) * 128],
                            latg[lc][:, cs],
                            start=(lc == 0), stop=(lc == LC - 1))
                for h in range(4):
                    nc.scalar.activation(qt[h][:, cs], psq[h][:], Ident,
                                         bias=bq[:, h:h + 1])
                for d in range(4):
                    nc.scalar.activation(kct[d][:, cs], psk_[d][:], Ident,
                                         bias=bk[:, d:d + 1])
                # wave 2: v (4 token-tiles) + pos_q (2) -> 6 banks
                psv = [psB.tile([128, 512], F32, name=f"psv{c}{i}",
                               tag=f"b{i}") for i in range(4)]
                pspq = [psB.tile([128, 512], F32, name=f"pspq{c}{p_}",
                                tag=f"b{4 + p_}") for p_ in range(2)]
                for lc in range(LC):
                    g, i = divmod(lc, 4)
                    for j in range(4):
                        tcol = c * 512 + j * 128
                        nc.tensor.matmul(
                            psv[j][:], latg[lc][:, tcol:tcol + 128],
                            wuv_r[g][:, i * 512:(i + 1) * 512],
                            start=(lc == 0), stop=False)
                    if lc < LQC:
                        for p_ in range(2):
                            nc.tensor.matmul(
                                pspq[p_][:],
                                wqp_r[:, lc * 256 + p_ * 128:
                                      lc * 256 + (p_ + 1) * 128],
                                latg[lc][:, cs],
                                start=(lc == 0), stop=(lc == LQC - 1))
                for j in range(4):
                    nc.tensor.matmul(psv[j][:], ones1[:], bv[:],
                                     start=False, stop=True)
                    nc.scalar.copy(vt[c * 4 + j][:], psv[j][:])
                for p_ in range(2):
                    raw = ropet.tile([128, 512], BF16, name=f"pqr{c}{p_}",
                                     tag="praw")
                    nc.scalar.activation(raw[:], pspq[p_][:], Ident,
                                         bias=bqp[:, p_:p_ + 1])
                    t1 = ropet.tile([128, 512], BF16, name=f"t1{c}{p_}",
                                    tag="t1")
                    tu = ropet.tile([128, 512], BF16, name=f"tu{c}{p_}",
                                    tag="tu")
                    tr = ropet.tile([128, 512], BF16, name=f"tr{c}{p_}",
                                    tag="tr")
                    nc.vector.tensor_mul(t1[:], raw[:], cosq[:, cs])
                    nc.vector.tensor_mul(tu[:], raw[:], sinq[:, cs])
                    for h2 in range(2):
                        o = h2 * 64
                        nc.sync.dma_start(tr[o:o + 32, :],
                                          tu[o + 32:o + 64, :])
                        nc.sync.dma_start(tr[o + 32:o + 64, :],
                                          tu[o:o + 32, :])
                    nc.vector.tensor_add(t1[:], t1[:], tr[:])
                    nc.gpsimd.dma_start(pq[2 * p_][:, cs], t1[0:64, :])
                    nc.gpsimd.dma_start(pq[2 * p_ + 1][:, cs],
                                        t1[64:128, :])

        wug0_ctx.close()

        # ---------------- Phase B2 ----------------
        with (
            tc.tile_pool(name="ep", bufs=4) as ep,
            tc.tile_pool(name="e8p", bufs=4) as e8p,
            tc.tile_pool(name="e2p", bufs=2) as e2p,
            tc.tile_pool(name="rcp", bufs=2) as rcp,
            tc.tile_pool(name="oep", bufs=3) as oep,
            tc.tile_pool(name="pss", bufs=3, space="PSUM") as pss,
            tc.tile_pool(name="psav", bufs=1, space="PSUM") as psav,
            tc.tile_pool(name="psden", bufs=1, space="PSUM") as psden,
        ):
            for qB in range(4):
                qs = slice(qB * 512, (qB + 1) * 512)
                npair = 2 * qB + 2
                fp8den = qB > 0   # qB=0 keeps bf16 den: few-key rows there
                for h in range(4):
                    av = psav.tile([128, 512], F32, name=f"av{h}{qB}",
                                   tag="av")
                    if fp8den:
                        den = psden.tile([64, 512], F32, name=f"dn{h}{qB}",
                                         tag="den")
                    else:
                        den = psden.tile([128, 512], F32, name=f"dn{h}{qB}",
                                         tag="den")
                    ee = []

                    def scores(p):
                        t = pss.tile([128, 1024], F32, name=f"s{h}{qB}{p}",
                                     tag="s")
                        for j in range(2):
                            kt = 2 * p + j
                            ks = slice(kt * 128, (kt + 1) * 128)
                            js = slice(j * 512, (j + 1) * 512)
                            nc.tensor.matmul(t[:, js], kct[h][:, ks],
                                             qt[h][:, qs],
                                             start=True, stop=False)
                            nc.tensor.matmul(t[:, js], pk[:, ks],
                                             pq[h][:, qs],
                                             start=False, stop=True)
                        e = ep.tile([128, 1024], BF16, name=f"e{h}{qB}{p}",
                                    tag="e")
                        nc.scalar.activation(e[:], t[:], Ex, scale=SCALE)
                        m = p - 2 * qB
                        if m >= 0:
                            e2 = e2p.tile([128, 1024], BF16,
                                          name=f"e2{h}{qB}{p}", tag="e2")
                            nc.vector.tensor_mul(e2[:], e[:], maskp[m][:])
                            e = e2
                        if fp8den:
                            e8 = e8p.tile([128, 1024], FP8,
                                          name=f"e8{h}{qB}{p}", tag="e8")
                            nc.vector.tensor_copy(e8[:], e[:])
                        else:
                            e8 = None
                        ee.append((e, e8))

                    def accum(p):
                        e, e8 = ee[p]
                        if fp8den:
                            for j in range(2):
                                # start clears the whole PSUM bank on hw:
                                # only the very first instr may set it
                                st = (p == 0 and j == 0)
                                sto = (p == npair - 1)
                                nc.tensor.matmul(
                                    den[:, j * 256:(j + 1) * 256],
                                    ONES8[:].rearrange(
                                        "p (i m) -> p i m", i=2),
                                    e8[:].rearrange(
                                        "p (i t) -> p i t",
                                        i=2)[:, :, j * 256:(j + 1) * 256],
                                    start=st, stop=sto, perf_mode=DR,
                                    skip_group_check=True)
                        for j in range(2):
                            kt = 2 * p + j
                            js = slice(j * 512, (j + 1) * 512)
                            st = (p == 0 and j == 0)
                            sto = (p == npair - 1 and j == 1)
                            if not fp8den:
                                nc.tensor.matmul(den[:], ONES[:], e[:, js],
                                                 start=st, stop=sto)
                            nc.tensor.matmul(
                                av[:], vt[kt][:, h * 128:(h + 1) * 128],
                                e[:, js], start=st, stop=sto)

                    # run scores two pairs ahead of accumulation to hide
                    # cross-engine semaphore latency
                    scores(0)
                    if npair > 1:
                        scores(1)
                    for p in range(2, npair):
                        scores(p)
                        accum(p - 2)
                    if npair > 1:
                        accum(npair - 2)
                    accum(npair - 1)
                    rc = rcp.tile([128, 512], F32, name=f"rc{h}{qB}",
                                  tag="rc")
                    if fp8den:
                        nc.vector.reciprocal_approx_fast(rc[0:64, :],
                                                         den[:])
                        nc.gpsimd.dma_start(rc[64:128, :], rc[0:64, :])
                    else:
                        nc.vector.reciprocal_approx_fast(rc[:], den[:])
                    nc.vector.tensor_mul(attn[h][:, qs], av[:], rc[:])

                # o_proj for this query block (borrows av+den banks)
                for mt in range(16):
                    pool = psav if mt % 2 == 0 else psden
                    tg = "av" if mt % 2 == 0 else "den"
                    op = pool.tile([128, 512], F32, name=f"op{qB}{mt}",
                                   tag=tg)
                    for hc in range(4):
                        nc.tensor.matmul(
                            op[:],
                            wos_r[hc][:, mt * 128:(mt + 1) * 128],
                            attn[hc][:, qs],
                            start=(hc == 0), stop=(hc == 3))
                    oe = oep.tile([128, 512], BF16, name=f"oe{qB}{mt}",
                                  tag="oe")
                    nc.scalar.copy(oe[:], op[:])
                    eng = nc.gpsimd if mt % 2 == 0 else nc.sync
                    eng.dma_start(T["OT"][mt * 128:(mt + 1) * 128, qs],
                                  oe[:])

        persist_ctx.close()


def build_program():
    nc = bacc.Bacc("TRN2", target_bir_lowering=False, debug=False,
                   num_devices=NCORES)
    T = {}

    def inp(name, shape, dt=BF16):
        T[name] = nc.dram_tensor(name, shape, dt, kind="ExternalInput").ap()

    inp("xT", [128, MC * TOK])
    inp("Wd", [128, 3 * MC * 512])
    inp("Wuq", [128, 3 * 2048])
    inp("Wuk", [128, 3 * 2048])
    inp("Wuv", [128, 3 * 2048])
    inp("Wqp", [128, 1024])
    inp("Wkp", [128, MC * PHD])
    inp("WoS", [512, MODEL])
    inp("cosq", [128, S])
    inp("sinq", [128, S])
    inp("cosk", [PHD, TOK])
    inp("sink", [PHD, TOK])
    inp("bd", [128, LC], F32)
    inp("bk", [128, 4], F32)
    inp("bq", [128, 4], F32)
    inp("bqp", [128, 2], F32)
    inp("bkp", [PHD, 1], F32)
    inp("bv", [1, 512])
    inp("maskp0", [128, 1024])
    inp("maskp1", [128, 1024])
    T["OT"] = nc.dram_tensor("OT", [MODEL, S], BF16,
                             kind="ExternalOutput").ap()

    with tile.TileContext(nc) as tc:
        _emit(nc, tc, T)
    nc.compile()
    return nc


def host_inputs(inputs):
    import ml_dtypes
    bf16 = ml_dtypes.bfloat16
    x = np.ascontiguousarray(np.asarray(inputs["x"], np.float32))
    W_down = np.asarray(inputs["W_down"], np.float32)
    b_down = np.asarray(inputs["b_down"], np.float32)
    W_up = np.asarray(inputs["W_up"], np.float32)
    b_up = np.asarray(inputs["b_up"], np.float32)
    W_qpos = np.asarray(inputs["W_qpos"], np.float32)
    b_qpos = np.asarray(inputs["b_qpos"], np.float32)
    W_kpos = np.asarray(inputs["W_kpos"], np.float32)
    b_kpos = np.asarray(inputs["b_kpos"], np.float32)
    W_o = np.asarray(inputs["W_o"], np.float32)

    inv = (1.0 / ROPE_THETA ** (np.arange(0, PHD, 2, dtype=np.float32) / PHD))
    t_all = np.arange(S, dtype=np.float32)
    fr = np.outer(inv, t_all)                       # [32, S]
    cc = np.concatenate([np.cos(fr), np.cos(fr)], 0)        # [64, S]
    ss = np.sin(fr)
    ssn = np.concatenate([ss, -ss], 0)                      # [64, S]
    cosq = np.vstack([cc, cc]).astype(np.float32)           # [128, S]
    sinq = np.vstack([ssn, ssn]).astype(np.float32)

    qq = np.arange(512)[None, :]
    kk = np.arange(128)[:, None]
    masks = [np.where(qq >= kk + m * 128, 1.0, 0.0).astype(np.float32)
             for m in range(4)]
    maskp0 = np.ascontiguousarray(np.concatenate([masks[0], masks[1]], 1))
    maskp1 = np.ascontiguousarray(np.concatenate([masks[2], masks[3]], 1))

    def pack_rows(w, nchunk):
        # [nchunk*128, F] -> [128, nchunk*F] with chunk-major columns
        F = w.shape[1]
        return np.ascontiguousarray(
            w.reshape(nchunk, 128, F).transpose(1, 0, 2).reshape(
                128, nchunk * F))

    def pack_wu(w):
        # [1536, 512] -> [128, 3*2048]: cols gg*2048 + i*512 + f
        return np.ascontiguousarray(
            w.reshape(3, 4, 128, 512).transpose(2, 0, 1, 3).reshape(
                128, 6144))

    common = {
        "Wd": np.ascontiguousarray(
            W_down.reshape(MC, 128, 3, 512).transpose(1, 2, 0, 3).reshape(
                128, 3 * MC * 512)),
        "Wkp": pack_rows(W_kpos, MC),
        "cosq": cosq, "sinq": sinq,
        "bd": np.ascontiguousarray(b_down.reshape(LC, 128).T),
        "bkp": np.ascontiguousarray(b_kpos[:, None]),
        "maskp0": maskp0, "maskp1": maskp1,
    }
    maps = []
    for c in range(NCORES):
        b, j = divmod(c, 4)
        ts = slice(j * TOK, (j + 1) * TOK)
        hs = slice(j * 512, (j + 1) * 512)
        m = dict(common)
        m["xT"] = pack_rows(np.ascontiguousarray(x[b, ts, :].T), MC)
        m["Wuq"] = pack_wu(W_up[:, :MODEL][:, hs])
        m["Wuk"] = pack_wu(W_up[:, MODEL:2 * MODEL][:, hs])
        m["Wuv"] = pack_wu(W_up[:, 2 * MODEL:][:, hs])
        m["Wqp"] = pack_rows(
            np.ascontiguousarray(W_qpos[:, j * 256:(j + 1) * 256]), 4)
        m["WoS"] = np.ascontiguousarray(W_o[hs, :])
        m["cosk"] = np.ascontiguousarray(cc[:, ts])
        m["sink"] = np.ascontiguousarray(ssn[:, ts])
        m["bq"] = np.ascontiguousarray(b_up[:MODEL][hs].reshape(4, 128).T)
        m["bk"] = np.ascontiguousarray(
            b_up[MODEL:2 * MODEL][hs].reshape(4, 128).T)
        m["bqp"] = np.ascontiguousarray(
            b_qpos[j * 256:(j + 1) * 256].reshape(2, 128).T)
        m["bv"] = np.ascontiguousarray(b_up[2 * MODEL:][hs][None, :])
        for key in list(m):
            if key not in F32_INPUTS:
                m[key] = np.ascontiguousarray(m[key]).astype(bf16)
        maps.append(m)
    return maps


_NC_CACHE = None


def _program():
    global _NC_CACHE
    if _NC_CACHE is None:
        _NC_CACHE = build_program()
    return _NC_CACHE


def kernel(**inputs) -> np.ndarray:
    nc = _program()
    maps = host_inputs(inputs)
    kwargs = {}
    if os.environ.get("BASSK_TRACE"):
        kwargs = dict(trace=True, trace_cores=list(range(NCORES)))
        td = os.environ.get("BASSK_TRACE_DIR")
        if td:
            kwargs["tmpdir"] = td
    res = bass_utils.run_bass_kernel_spmd(
        nc, maps, core_ids=list(range(NCORES)), **kwargs)
    kernel.last_results = res
    b_o = np.asarray(inputs["b_o"], np.float32)
    out = np.empty((B, S, MODEL), np.float32)
    for b in range(B):
        acc = np.asarray(res.results[b * 4]["OT"], np.float32)
        for c in range(b * 4 + 1, b * 4 + 4):
            acc = acc + np.asarray(res.results[c]["OT"], np.float32)
        out[b] = acc.T + b_o[None, :]
    return out
